# revision 1
# baseline (speedup 1.0000x reference)
"""Trainium2 Bass kernel for nn_AttentionLayer_13383118095164.

Cross-attention layer: q = target @ Wq + bq; k/v = source/value @ Wk/Wv + bk/bv;
out = softmax(q k^T / 8) v @ Wo + bo.   B=4, L=2048, S=1024, D=1024, H=16, E=64.

Sharding (8 cores): core c = (batch b = c//2, head-group g = c%2 of 8 heads).
Megatron-style: Q/K/V column-split by head group, Wo row-split; the two
head-group partial outputs per batch are summed on the host.

Design notes (v1 baseline was 752us; this version measures ~453us):
  * All matmul operands bf16 (host-cast): halves HBM traffic (60MB -> 30MB
    per core) and SBUF footprint; PE rate unchanged (bf16 = full rate).
    PSUM accumulation stays fp32, tolerance is 2e-2 so ~0.5% bf16 error is fine.
  * xt/wq/wo resident in SBUF (loaded during the K-proj stream), so Q
    projection becomes per-(lc,mt) granules usable as PE filler.
  * Attention phase restructured around the real bottleneck (ACT exp at
    1 elem/cycle/lane = ~2.3us per score batch): PSUM is budgeted as
    sc0+sc1 bigs (scores, 1 batch) + av big (2-head O^T accum) + 2 misc
    half-bigs (Q granules / out-proj groups / rcp broadcasts), and the PE
    gaps while ACT drains scores are filled with Q-proj granules and the
    previous l-chunk's out-projection groups instead of idling.
  Device layout identical to v1 otherwise: everything transposed, softmax
  denominator via a ones-column in V (M=65 AV matmuls), bk dropped
  (softmax shift invariance), bv@Wo+bo folded into a host constant,
  bq and the 1/8 scale fused into the Q copyback activation.
"""

import numpy as np
import ml_dtypes

import concourse.bass as bass
import concourse.tile as tile
from concourse import mybir
from concourse.bass_utils import run_bass_kernel_spmd

P = 128
D = 1024  # d_model
DL = 4096  # d_llm
L = 2048  # target length
S = 1024  # source length
MQ = 512  # per-core q/k/v dims (8 heads x 64)
E = 64
E1 = E + 1
HG = 8  # heads per core
LC = 512  # l-chunk
N_LC = L // LC
SCALE = 0.125  # 1/sqrt(E)

BF16 = mybir.dt.bfloat16
F32 = mybir.dt.float32


def _split_multi_waits(nc):
    """This walrus build rejects >1 sync wait per instruction: split extras
    onto single-wait NOPs on the same engine immediately before (same program
    order on the same queue => identical semantics)."""
    for f in nc.m.functions:
        for blk in f.blocks:
            new_insts = []
            for inst in blk.instructions:
                si = inst.sync_info
                if si is not None and si.on_wait and len(si.on_wait) > 1:
                    waits = list(si.on_wait)
                    for w in waits[:-1]:
                        nop = mybir.InstNoOp(
                            name=f"I-waitsplit-{nc.next_id()}", ins=[], outs=[]
                        )
                        nop.engine = inst.engine
                        nop.sync_info = mybir.SyncInfo(on_wait=[w], on_update=[])
                        new_insts.append(nop)
                    si.on_wait = [waits[-1]]
                new_insts.append(inst)
            blk.instructions[:] = new_insts


def build_nc(reps: int = 1, timing: bool = False):
    """timing=True: identical device program, but all real tensors are
    Internal DRAM (uninitialized) with tiny dummy external I/O, so timing
    runs ship no host data and the slope measurement is low-variance. The
    engines here are data-independent in speed, so per-rep time is the same."""
    from contextlib import ExitStack

    nc = bass.Bass(trn_type="TRN2", target_bir_lowering=False, debug=False)

    ikind = "Internal" if timing else "ExternalInput"
    okind = "Internal" if timing else "ExternalOutput"
    xt = nc.dram_tensor("xt", [D, L], BF16, kind=ikind)  # X_t^T
    xs = nc.dram_tensor("xs", [DL, S], BF16, kind=ikind)  # X_s^T
    xv = nc.dram_tensor("xv", [DL, S], BF16, kind=ikind)  # X_v^T
    wq = nc.dram_tensor("wq", [D, MQ], BF16, kind=ikind)
    wk = nc.dram_tensor("wk", [DL, MQ], BF16, kind=ikind)
    wv = nc.dram_tensor("wv", [DL, MQ], BF16, kind=ikind)
    wo = nc.dram_tensor("wo", [MQ, D], BF16, kind=ikind)
    bq = nc.dram_tensor("bq", [P, 4], F32, kind=ikind)  # bq/8 as [p, mt]
    out = nc.dram_tensor("out", [D, L], F32, kind=okind)  # out^T partial
    if timing:
        dummy = nc.dram_tensor("tdin", [1, 4], F32, kind="ExternalInput")
        dumout = nc.dram_tensor("tdout", [1, 4], F32, kind="ExternalOutput")

    with tile.TileContext(nc) as tc, ExitStack() as ctx:
        const = ctx.enter_context(tc.tile_pool(name="const", bufs=1))
        resident = ctx.enter_context(tc.tile_pool(name="resident", bufs=1))
        stream = ctx.enter_context(tc.tile_pool(name="stream", bufs=6))
        stream2 = ctx.enter_context(tc.tile_pool(name="stream2", bufs=6))
        psum = ctx.enter_context(tc.tile_pool(name="psum", bufs=1, space="PSUM"))
        psum2 = ctx.enter_context(tc.tile_pool(name="psum2", bufs=2, space="PSUM"))

        # ---- constants ----
        if timing:
            dtile = const.tile([1, 4], F32, name="dtile")
            nc.sync.dma_start(dtile[:], dummy.ap())
            nc.sync.dma_start(dumout.ap(), dtile[:])
        bq_sb = const.tile([P, 4], F32, name="bq_sb")
        nc.sync.dma_start(bq_sb[:], bq.ap())
        ones64 = const.tile([1, E], BF16, name="ones64")
        nc.vector.memset(ones64[:], 1.0)

        # ---- residents ----
        kT = resident.tile([P, 4, S], BF16, name="kT")  # [p, mt, s]
        v_sb = resident.tile([P, 8, HG, E1], BF16, name="v_sb")  # [p, st, h, e|1]
        nc.vector.memset(v_sb[:, :, :, E : E + 1], 1.0)
        qT = resident.tile([P, 4, N_LC, LC], BF16, name="qT")  # [p, mt, lc, l]
        xt_sb = resident.tile([P, D // P, L], BF16, name="xt_sb")  # [p, kt, l]
        wq_sb = resident.tile([P, D // P, MQ], BF16, name="wq_sb")
        wo_sb = resident.tile([P, MQ // P, D], BF16, name="wo_sb")

        def _body(work):
            _emit_rep(
                nc, psum, psum2, work, stream, stream2,
                xt, xs, xv, wq, wk, wv, wo,
                bq_sb, ones64, kT, v_sb, qT, xt_sb, wq_sb, wo_sb, out,
            )

        if reps == 1:
            with tc.tile_pool(name="work", bufs=4) as work:
                _body(work)
        else:
            # Unroll 2 reps per For_i iteration: hardware-loop iterations
            # cannot overlap, but Tile schedules across the two unrolled
            # bodies, so half the rep boundaries pipeline (DMA of the next
            # rep's K stream under the previous rep's attention tail).
            assert reps % 2 == 0 or reps == 1, "reps must be even"
            with tc.For_i(0, reps // 2, 1):
                with tc.tile_pool(name="work", bufs=4) as work:
                    _body(work)
                    _body(work)

    _split_multi_waits(nc)
    return nc


def _emit_rep(nc, psum, psum2, work, stream, stream2,
              xt, xs, xv, wq, wk, wv, wo,
              bq_sb, ones64, kT, v_sb, qT, xt_sb, wq_sb, wo_sb, out):
    names = [0]

    def uid(s):
        names[0] += 1
        return f"{s}_{names[0]}"

    def alloc8(pfx):
        """All 8 PSUM banks as 8 [P,512] accumulators."""
        b0 = psum.tile([P, 1024], F32, tag="sc0", name=uid(f"{pfx}b0"))
        b1 = psum.tile([P, 1024], F32, tag="sc1", name=uid(f"{pfx}b1"))
        b2 = psum.tile([P, 1024], F32, tag="av", name=uid(f"{pfx}b2"))
        m0 = psum2.tile([P, 512], F32, tag="misc", name=uid(f"{pfx}m0"))
        m1 = psum2.tile([P, 512], F32, tag="misc", name=uid(f"{pfx}m1"))
        return [
            b0[:, 0:512], b0[:, 512:1024],
            b1[:, 0:512], b1[:, 512:1024],
            b2[:, 0:512], b2[:, 512:1024],
            m0[:], m1[:],
        ]

    # ---------- Phase A: K^T = Wk_g^T @ X_s^T -> kT[p, mt, s] ----------
    # Resident loads for later phases ride along the stream so they are
    # fully hidden behind the K-proj PE work.
    kacc = alloc8("k")
    for kt in range(DL // P):
        wk_t = stream.tile([P, MQ], BF16, tag="wk_t", name=uid("wk_t"))
        nc.sync.dma_start(wk_t[:], wk.ap()[kt * P : (kt + 1) * P, :])
        xs_t = stream2.tile([P, S], BF16, tag="xs_t", name=uid("xs_t"))
        nc.sync.dma_start(xs_t[:], xs.ap()[kt * P : (kt + 1) * P, :])
        if kt >= 2 and kt % 3 == 2 and (kt - 2) // 3 < 8:
            j = (kt - 2) // 3
            nc.sync.dma_start(xt_sb[:, j, :], xt.ap()[j * P : (j + 1) * P, :])
        elif kt == 27:
            nc.sync.dma_start(
                wq_sb[:], wq.ap().rearrange("(kt p) m -> p kt m", p=P)
            )
        elif kt == 30:
            nc.sync.dma_start(
                wo_sb[:], wo.ap().rearrange("(kt p) d -> p kt d", p=P)
            )
        for mt in range(4):
            for sc in range(2):
                nc.tensor.matmul(
                    kacc[mt * 2 + sc],
                    wk_t[:, mt * P : (mt + 1) * P],
                    xs_t[:, sc * 512 : (sc + 1) * 512],
                    start=(kt == 0),
                    stop=(kt == DL // P - 1),
                )
    with nc.allow_low_precision(reason="bf16 operands keep ~8 mantissa bits; "
                                "rel tolerance is 2e-2"):
        for i in range(8):
            nc.vector.tensor_copy(
                kT[:, i // 2, (i % 2) * 512 : (i % 2 + 1) * 512], kacc[i]
            )

    # ---------- Phase Q0: Q proj for lc=0 ----------
    def q_granule(lc, mt):
        qm = psum2.tile([P, 512], F32, tag="misc", name=uid("qm"))
        for kt in range(D // P):
            nc.tensor.matmul(
                qm[:],
                wq_sb[:, kt, mt * P : (mt + 1) * P],
                xt_sb[:, kt, lc * LC : (lc + 1) * LC],
                start=(kt == 0),
                stop=(kt == D // P - 1),
            )
        with nc.allow_low_precision(reason="bf16 q keeps ~8 mantissa bits"):
            nc.scalar.activation(
                qT[:, mt, lc, :],
                qm[:],
                mybir.ActivationFunctionType.Identity,
                bias=bq_sb[:, mt : mt + 1],
                scale=SCALE,
            )

    for mt in range(4):
        q_granule(0, mt)

    # ---------- Phase B: V = X_v @ Wv_g -> v_sb[p, st, h, 0:64] ----------
    vacc = alloc8("v")
    for kt in range(DL // P):
        wv_t = stream.tile([P, MQ], BF16, tag="wk_t", name=uid("wv_t"))
        nc.sync.dma_start(wv_t[:], wv.ap()[kt * P : (kt + 1) * P, :])
        xv_t = stream2.tile([P, S], BF16, tag="xs_t", name=uid("xv_t"))
        nc.sync.dma_start(xv_t[:], xv.ap()[kt * P : (kt + 1) * P, :])
        for st in range(8):
            nc.tensor.matmul(
                vacc[st],
                xv_t[:, st * P : (st + 1) * P],
                wv_t[:, :],
                start=(kt == 0),
                stop=(kt == DL // P - 1),
            )
    with nc.allow_low_precision(reason="bf16 v keeps ~8 mantissa bits"):
        for st in range(8):
            nc.vector.tensor_copy(
                v_sb[:, st, :, 0:E], vacc[st].rearrange("p (h e) -> p h e", e=E)
            )

    # ---------- Phase C: attention, ACT-paced with PE filler ----------
    # Batch = (lc, hp, g, stp): 4 score MMs (2 row-paired slots) -> sc0/sc1,
    # 2 exps -> expS (bf16). AV for the previous batch trails; Q granules for
    # lc+1 and out-proj groups for lc-1 fill the PE while ACT drains scores.
    oTs = {}

    def issue_scores(lc, hp, g, stp, expS):
        for a in range(2):
            t = psum.tile([P, 1024], F32, tag=f"sc{a}", name=uid("sc"))
            for half in range(2):
                st = 4 * g + 2 * stp + half
                pa = 64 * a
                nc.tensor.matmul(
                    t[:, 512 * half : 512 * (half + 1)],
                    kT[pa : pa + 64, hp, st * P : (st + 1) * P],
                    qT[pa : pa + 64, hp, lc, :],
                    start=True,
                    stop=True,
                )
            with nc.allow_low_precision(reason="bf16 probs keep ~8 mantissa "
                                        "bits; tolerance 2e-2"):
                nc.scalar.activation(
                    expS[:, 2 * a : 2 * a + 2, :],
                    t[:].rearrange("p (t l) -> p t l", l=LC),
                    mybir.ActivationFunctionType.Exp,
                )

    av_tiles = {}  # (lc, hp) -> av big tile

    def issue_av(lc, hp, g, stp, expS):
        if g == 0 and stp == 0:
            av_tiles[(lc, hp)] = psum.tile(
                [P, 1024], F32, tag="av", name=uid("av")
            )
        av = av_tiles[(lc, hp)]
        for a in range(2):
            h = 2 * hp + a
            po = av[:, 512 * a : 512 * a + 512]
            for half in range(2):
                st = 4 * g + 2 * stp + half
                nc.tensor.matmul(
                    po[0:E1, :],
                    v_sb[:, st, h, :],
                    expS[:, 2 * a + half, :],
                    start=(st == 0),
                    stop=(st == 7),
                    skip_group_check=True,
                )
        if g == 1 and stp == 1:
            finalize_hp(lc, hp)

    def finalize_hp(lc, hp):
        if lc not in oTs:
            oTs[lc] = work.tile([P, 4, LC], BF16, tag="oT", name=uid("oT"))
        oT = oTs[lc]
        av = av_tiles.pop((lc, hp))
        # One reciprocal over both heads' denominator rows, then two K=1
        # col-tiled broadcasts into disjoint 64-partition ranges of one pb
        # tile, one bsb copy, two muls.
        rcpv = work.tile([1, 2, LC], BF16, tag="rcp", name=uid("rcp"))
        with nc.allow_low_precision(reason="denominator reciprocal in "
                                    "bf16; tolerance 2e-2"):
            nc.vector.reciprocal(
                rcpv[:].rearrange("p t l -> p (t l)"), av[E : E + 1, :]
            )
        pb = psum2.tile([P, 512], F32, tag="misc", name=uid("pb"))
        for a in range(2):
            nc.tensor.matmul(
                pb[64 * a : 64 * a + 64, :], ones64[:], rcpv[0:1, a, :],
                start=True, stop=True,
            )
        bsb = work.tile([P, LC], F32, tag="bsb", name=uid("bsb"))
        nc.vector.tensor_copy(bsb[:], pb[:])
        with nc.allow_low_precision(reason="bf16 attention output; "
                                    "tolerance 2e-2"):
            for a in range(2):
                nc.vector.tensor_mul(
                    oT[64 * a : 64 * a + 64, hp, :],
                    av[0:E, 512 * a : 512 * a + 512],
                    bsb[64 * a : 64 * a + 64, :],
                )

    def outproj_group(lc, mt8):
        oT = oTs[lc]
        og = psum2.tile([P, 512], F32, tag="misc", name=uid("og"))
        for kt4 in range(4):
            nc.tensor.matmul(
                og[:],
                wo_sb[:, kt4, mt8 * P : (mt8 + 1) * P],
                oT[:, kt4, :],
                start=(kt4 == 0),
                stop=(kt4 == 3),
            )
        stg = work.tile([P, LC], F32, tag="stg", name=uid("stg"))
        nc.vector.tensor_copy(stg[:], og[:])
        nc.sync.dma_start(
            out.ap()[mt8 * P : (mt8 + 1) * P, lc * LC : (lc + 1) * LC], stg[:]
        )
        if mt8 == 7:
            del oTs[lc]

    batches = [
        (lc, hp, g, stp)
        for lc in range(N_LC)
        for hp in range(4)
        for g in range(2)
        for stp in range(2)
    ]
    # PE program order per batch k: [AV(k-2)] [filler] [scores(k)+exp(k)].
    # AV runs 2 batches behind so its exp is long done when the PE reaches
    # it; scores(k) stalls only on exp(k-1) freeing the sc PSUM slot, which
    # is the intended ACT pacing, with the PE queue already drained of work.
    from collections import deque

    pend = deque()  # (lc, hp, g, stp, expS_pair), depth 2
    for bi, (lc, hp, g, stp) in enumerate(batches):
        if len(pend) == 2:
            issue_av(*pend.popleft())
        # filler: Q granules for lc+1 early in this lc; out-proj for lc-1
        within = bi % 16
        if lc < N_LC - 1 and within in (1, 3, 5, 7):
            q_granule(lc + 1, (within - 1) // 2)
        if lc > 0 and within in (2, 4, 6, 8, 10, 12, 14, 15):
            mt8 = (2, 4, 6, 8, 10, 12, 14, 15).index(within)
            outproj_group(lc - 1, mt8)
        expS = work.tile([P, 4, LC], BF16, tag="expS", name=uid("e"))
        issue_scores(lc, hp, g, stp, expS)
        pend.append((lc, hp, g, stp, expS))
    # drain tail
    while pend:
        issue_av(*pend.popleft())
    for mt8 in range(8):
        outproj_group(N_LC - 1, mt8)


_NC_CACHE = {}


def _get_nc(reps=1, timing=False):
    if (reps, timing) not in _NC_CACHE:
        _NC_CACHE[(reps, timing)] = build_nc(reps, timing)
    return _NC_CACHE[(reps, timing)]


def make_in_maps(inputs):
    bf = ml_dtypes.bfloat16
    te = np.asarray(inputs["target_embedding"], np.float32)
    se = np.asarray(inputs["source_embedding"], np.float32)
    ve = np.asarray(inputs["value_embedding"], np.float32)
    Wq = np.asarray(inputs["Wq"], np.float32)
    Wk = np.asarray(inputs["Wk"], np.float32)
    Wv = np.asarray(inputs["Wv"], np.float32)
    Wo = np.asarray(inputs["Wo"], np.float32)
    bqv = np.asarray(inputs["bq"], np.float32)
    in_maps = []
    for core in range(8):
        b, g = divmod(core, 2)
        sl = slice(MQ * g, MQ * (g + 1))
        in_maps.append(
            {
                "xt": np.ascontiguousarray(te[b].T).astype(bf),
                "xs": np.ascontiguousarray(se[b].T).astype(bf),
                "xv": np.ascontiguousarray(ve[b].T).astype(bf),
                "wq": np.ascontiguousarray(Wq[:, sl]).astype(bf),
                "wk": np.ascontiguousarray(Wk[:, sl]).astype(bf),
                "wv": np.ascontiguousarray(Wv[:, sl]).astype(bf),
                "wo": np.ascontiguousarray(Wo[sl, :]).astype(bf),
                "bq": np.ascontiguousarray((bqv[sl] * SCALE).reshape(4, P).T),
            }
        )
    return in_maps


def assemble_output(results, inputs):
    bv = np.asarray(inputs["bv"], np.float32)
    bo = np.asarray(inputs["bo"], np.float32)
    Wo = np.asarray(inputs["Wo"], np.float32)
    corr = (bv @ Wo + bo).astype(np.float32)  # [D]
    out = np.empty((4, L, D), np.float32)
    for b in range(4):
        acc = results[2 * b]["out"] + results[2 * b + 1]["out"]  # [D, L]
        out[b] = np.asarray(acc, np.float32).T + corr
    return out


def _run_once(nc, in_maps, inputs):
    last_err = None
    for _attempt in range(3):
        try:
            res = run_bass_kernel_spmd(nc, in_maps, core_ids=list(range(8)))
            return assemble_output(res.results, inputs)
        except Exception as e:  # transient NRT device errors: retry
            last_err = e
    raise last_err


def kernel(**inputs) -> np.ndarray:
    nc = _get_nc(1)
    in_maps = make_in_maps(inputs)
    # Run twice and require agreement: guards against rare transient silent
    # device corruption (observed once after an abnormal device state).
    outs = [_run_once(nc, in_maps, inputs) for _ in range(2)]
    for _extra in range(2):
        scale = float(np.abs(outs[-1]).mean()) + 1e-30
        if any(
            np.abs(o - outs[-1]).max() < 1e-3 * scale for o in outs[:-1]
        ):
            return outs[-1]
        outs.append(_run_once(nc, in_maps, inputs))
    return outs[-1]



# revision 20
# speedup vs baseline: 1.1065x; 1.1065x over previous
"""Trainium2 Bass kernel for nn_AttentionLayer_13383118095164.

Cross-attention layer: q = target @ Wq + bq; k/v = source/value @ Wk/Wv + bk/bv;
out = softmax(q k^T / 8) v @ Wo + bo.   B=4, L=2048, S=1024, D=1024, H=16, E=64.

Sharding (8 cores): core c = (batch b = c//2, head-group g = c%2 of 8 heads).
Megatron-style: Q/K/V column-split by head group, Wo row-split; the two
head-group partial outputs per batch are summed on the host.

v3 design notes (baseline 475us this session, v2 449us, v3 ~434us; HW
microbenches drove each rev):
  * DMA: a single HWDGE queue sustains only ~148GB/s (measured); the 30MB
    of input traffic is split across the two HWDGE queues (SP +
    Activation), which run in parallel at ~148GB/s each.
  * ACT: exp costs ~1.3us per [128,2,512] PSUM-sourced instruction but only
    ~1.65us for a 4-bank [128,4,512] one (per-inst overhead dominates), so
    scores per batch land in ONE [P,2048] PSUM tile (4 banks) consumed by
    ONE exp.  PSUM: sc(4) + av(2) + misc(2) banks.
  * Scores are issued FIRST in each batch's PE program (they gate the next
    exp; AV + fillers follow) and the two 64-row PE tiles (head pair) are
    interleaved so they run concurrently (64x128 row tiling, measured
    118ns/MM vs 150 sequential).
  * Q-projection copyback moved from ACT (the attention pacer) to DVE
    (tensor_scalar mul+add).
  * out written bf16 (halves output DMA); host sums partials in fp32.
  * Timing loop unrolled 4x per For_i iteration (each hardware-loop
    boundary costs ~2.9us of drain, measured).
  Rejected after measuring slower: DVE evacuation of scores to SBUF for a
  cheaper SBUF-sourced exp (DVE queueing delayed exp more than the PSUM
  read penalty it saved: 490us vs 434us).
  Device layout otherwise as v1/v2: everything transposed, softmax
  denominator via a ones-column in V (M=65 AV matmuls), bk dropped
  (softmax shift invariance), bv@Wo+bo folded into a host constant.
"""

import numpy as np
import ml_dtypes

import concourse.bass as bass
import concourse.tile as tile
from concourse import mybir
from concourse.bass_utils import run_bass_kernel_spmd

P = 128
D = 1024  # d_model
DL = 4096  # d_llm
L = 2048  # target length
S = 1024  # source length
MQ = 512  # per-core q/k/v dims (8 heads x 64)
E = 64
E1 = E + 1
HG = 8  # heads per core
LC = 512  # l-chunk
N_LC = L // LC
SCALE = 0.125  # 1/sqrt(E)

BF16 = mybir.dt.bfloat16
F32 = mybir.dt.float32


def _split_multi_waits(nc):
    """This walrus build rejects >1 sync wait per instruction: split extras
    onto single-wait NOPs on the same engine immediately before (same program
    order on the same queue => identical semantics)."""
    for f in nc.m.functions:
        for blk in f.blocks:
            new_insts = []
            for inst in blk.instructions:
                si = inst.sync_info
                if si is not None and si.on_wait and len(si.on_wait) > 1:
                    waits = list(si.on_wait)
                    for w in waits[:-1]:
                        nop = mybir.InstNoOp(
                            name=f"I-waitsplit-{nc.next_id()}", ins=[], outs=[]
                        )
                        nop.engine = inst.engine
                        nop.sync_info = mybir.SyncInfo(on_wait=[w], on_update=[])
                        new_insts.append(nop)
                    si.on_wait = [waits[-1]]
                new_insts.append(inst)
            blk.instructions[:] = new_insts


def build_nc(reps: int = 1, timing: bool = False, phases: str = "all"):
    """timing=True: identical device program, but all real tensors are
    Internal DRAM (uninitialized) with tiny dummy external I/O, so timing
    runs ship no host data and the slope measurement is low-variance. The
    engines here are data-independent in speed, so per-rep time is the same."""
    from contextlib import ExitStack

    nc = bass.Bass(trn_type="TRN2", target_bir_lowering=False, debug=False)

    ikind = "Internal" if timing else "ExternalInput"
    okind = "Internal" if timing else "ExternalOutput"
    xt = nc.dram_tensor("xt", [D, L], BF16, kind=ikind)  # X_t^T
    xs = nc.dram_tensor("xs", [DL, S], BF16, kind=ikind)  # X_s^T
    xv = nc.dram_tensor("xv", [DL, S], BF16, kind=ikind)  # X_v^T
    wq = nc.dram_tensor("wq", [D, MQ], BF16, kind=ikind)
    wk = nc.dram_tensor("wk", [DL, MQ], BF16, kind=ikind)
    wv = nc.dram_tensor("wv", [DL, MQ], BF16, kind=ikind)
    wo = nc.dram_tensor("wo", [MQ, D], BF16, kind=ikind)
    bq = nc.dram_tensor("bq", [P, 4], F32, kind=ikind)  # bq/8 as [p, mt]
    out = nc.dram_tensor("out", [D, L], BF16, kind=okind)  # out^T partial
    if timing:
        dummy = nc.dram_tensor("tdin", [1, 4], F32, kind="ExternalInput")
        dumout = nc.dram_tensor("tdout", [1, 4], F32, kind="ExternalOutput")

    with tile.TileContext(nc) as tc, ExitStack() as ctx:
        const = ctx.enter_context(tc.tile_pool(name="const", bufs=1))
        resident = ctx.enter_context(tc.tile_pool(name="resident", bufs=1))
        stream = ctx.enter_context(tc.tile_pool(name="stream", bufs=6))
        stream2 = ctx.enter_context(tc.tile_pool(name="stream2", bufs=6))
        psum = ctx.enter_context(tc.tile_pool(name="psum", bufs=2, space="PSUM"))
        psumav = ctx.enter_context(tc.tile_pool(name="psumav", bufs=1, space="PSUM"))
        psum2 = ctx.enter_context(tc.tile_pool(name="psum2", bufs=1, space="PSUM"))

        # ---- constants ----
        if timing:
            dtile = const.tile([1, 4], F32, name="dtile")
            nc.sync.dma_start(dtile[:], dummy.ap())
            nc.sync.dma_start(dumout.ap(), dtile[:])
        bq_sb = const.tile([P, 4], F32, name="bq_sb")
        nc.sync.dma_start(bq_sb[:], bq.ap())
        ones128 = const.tile([P, E], BF16, name="ones128")
        nc.vector.memset(ones128[:], 1.0)

        # ---- residents ----
        kT = resident.tile([P, 4, S], BF16, name="kT")  # [p, mt, s]
        v_sb = resident.tile([P, 8, HG, E], BF16, name="v_sb")  # [p, st, h, e]
        esump = ctx.enter_context(tc.tile_pool(name="esump", bufs=2))
        expp = ctx.enter_context(tc.tile_pool(name="expp", bufs=4))
        qT = resident.tile([P, 4, N_LC, LC], BF16, name="qT")  # [p, mt, lc, l]
        xt_sb = resident.tile([P, D // P, L], BF16, name="xt_sb")  # [p, kt, l]
        wq_sb = resident.tile([P, D // P, MQ], BF16, name="wq_sb")
        wo_sb = resident.tile([P, MQ // P, D], BF16, name="wo_sb")

        def _body(work):
            _emit_rep(
                nc, psum, psumav, psum2, esump, expp, work, stream, stream2,
                xt, xs, xv, wq, wk, wv, wo,
                bq_sb, ones128, kT, v_sb, qT, xt_sb, wq_sb, wo_sb, out,
                phases,
            )

        if reps == 1:
            with tc.tile_pool(name="work", bufs=4) as work:
                _body(work)
        else:
            # Unroll reps per For_i iteration: hardware-loop iterations
            # cannot overlap, but Tile schedules across the unrolled bodies,
            # so rep boundaries inside an iteration pipeline.
            unroll = 4 if reps % 4 == 0 else 2
            assert reps % 2 == 0 or reps == 1, "reps must be even"
            with tc.For_i(0, reps // unroll, 1):
                with tc.tile_pool(name="work", bufs=4) as work:
                    for _ in range(unroll):
                        _body(work)

    _split_multi_waits(nc)
    return nc


def _emit_rep(nc, psum, psumav, psum2, esump, expp, work, stream, stream2,
              xt, xs, xv, wq, wk, wv, wo,
              bq_sb, ones128, kT, v_sb, qT, xt_sb, wq_sb, wo_sb, out,
              phases="all"):
    names = [0]
    do_ab = phases in ("all", "ab")
    do_attn = phases in ("all", "attn")

    def uid(s):
        names[0] += 1
        return f"{s}_{names[0]}"

    def alloc8(pfx):
        """All 8 PSUM banks as 8 [P,512] accumulators (2 sc + av + misc)."""
        b0 = psum.tile([P, 1536], F32, tag="sc", name=uid(f"{pfx}b0"))
        b1 = psum.tile([P, 1536], F32, tag="sc", name=uid(f"{pfx}b1"))
        a0 = psumav.tile([P, 512], F32, tag="av", name=uid(f"{pfx}a0"))
        m0 = psum2.tile([P, 512], F32, tag="misc", name=uid(f"{pfx}m0"))
        return [
            b0[:, 0:512], b0[:, 512:1024], b0[:, 1024:1536],
            b1[:, 0:512], b1[:, 512:1024], b1[:, 1024:1536],
            a0[:],
            m0[:],
        ]

    def q_granule(lc, mt):
        qm = psum2.tile([P, 512], F32, tag="misc", name=uid("qm"))
        for kt in range(D // P):
            nc.tensor.matmul(
                qm[:],
                wq_sb[:, kt, mt * P : (mt + 1) * P],
                xt_sb[:, kt, lc * LC : (lc + 1) * LC],
                start=(kt == 0),
                stop=(kt == D // P - 1),
            )
        with nc.allow_low_precision(reason="bf16 q keeps ~8 mantissa bits"):
            nc.vector.tensor_scalar(
                qT[:, mt, lc, :],
                qm[:],
                SCALE,
                bq_sb[:, mt : mt + 1],
                mybir.AluOpType.mult,
                mybir.AluOpType.add,
            )

    # ---------- Phase A: K^T = Wk_g^T @ X_s^T -> kT[p, mt, s] ----------
    # wk + residents ride the SP queue; xs streams on the ACT HWDGE queue
    # (idle during projections) so the two queues halve the DMA wall time.
    if do_ab:
        kacc = alloc8("k")
        for kt in range(DL // P):
            wk_t = stream.tile([P, MQ], BF16, tag="wk_t", name=uid("wk_t"))
            nc.sync.dma_start(wk_t[:], wk.ap()[kt * P : (kt + 1) * P, :])
            xs_t = stream2.tile([P, S], BF16, tag="xs_t", name=uid("xs_t"))
            nc.scalar.dma_start(xs_t[:], xs.ap()[kt * P : (kt + 1) * P, :])
            if kt >= 2 and kt % 3 == 2 and (kt - 2) // 3 < 8:
                j = (kt - 2) // 3
                nc.sync.dma_start(xt_sb[:, j, :], xt.ap()[j * P : (j + 1) * P, :])
            elif kt == 27:
                nc.sync.dma_start(
                    wq_sb[:], wq.ap().rearrange("(kt p) m -> p kt m", p=P)
                )
            elif kt == 30:
                nc.sync.dma_start(
                    wo_sb[:], wo.ap().rearrange("(kt p) d -> p kt d", p=P)
                )
            for mt in range(4):
                for sc in range(2):
                    nc.tensor.matmul(
                        kacc[mt * 2 + sc],
                        wk_t[:, mt * P : (mt + 1) * P],
                        xs_t[:, sc * 512 : (sc + 1) * 512],
                        start=(kt == 0),
                        stop=(kt == DL // P - 1),
                    )
        with nc.allow_low_precision(reason="bf16 operands keep ~8 mantissa "
                                    "bits; rel tolerance is 2e-2"):
            for i in range(8):
                nc.vector.tensor_copy(
                    kT[:, i // 2, (i % 2) * 512 : (i % 2 + 1) * 512], kacc[i]
                )

        # ---------- Phase Q0: Q proj for lc=0 ----------
        for mt in range(4):
            q_granule(0, mt)

        # ---------- Phase B: V = X_v @ Wv_g -> v_sb[p, st, h, 0:64] --------
        vacc = alloc8("v")
        for kt in range(DL // P):
            wv_t = stream.tile([P, MQ], BF16, tag="wk_t", name=uid("wv_t"))
            nc.sync.dma_start(wv_t[:], wv.ap()[kt * P : (kt + 1) * P, :])
            xv_t = stream2.tile([P, S], BF16, tag="xs_t", name=uid("xv_t"))
            # alternate xv between the two HWDGE queues to balance load
            xv_q = nc.scalar if kt % 2 == 0 else nc.sync
            xv_q.dma_start(xv_t[:], xv.ap()[kt * P : (kt + 1) * P, :])
            for st in range(8):
                nc.tensor.matmul(
                    vacc[st],
                    xv_t[:, st * P : (st + 1) * P],
                    wv_t[:, :],
                    start=(kt == 0),
                    stop=(kt == DL // P - 1),
                )
        with nc.allow_low_precision(reason="bf16 v keeps ~8 mantissa bits"):
            for st in range(8):
                nc.vector.tensor_copy(
                    v_sb[:, st, :, 0:E],
                    vacc[st].rearrange("p (h e) -> p h e", e=E),
                )

    if not do_attn:
        return

    # ---------- Phase C: attention, slab pipeline ----------
    # Unit of work = one score slab (lc, hp, st, a): [128 s, 512 l] for one
    # head-half.  Slabs stream st-major within each (lc, hp) unit; groups of
    # 3 consecutive slabs share one [P,1536] sc tile (3 banks) and ONE exp.
    # sc rotates x2 (6 banks) so exp(k+1) never waits on bank recycling:
    # ACT runs back-to-back.  AV is col-tiled (M=64, heads stacked on PSUM
    # partitions) into a 1-bank av tile; softmax denominators come from DVE
    # slab-sums (fp32) reduced+broadcast by a ones-stationary matmul.
    oTs = {}
    av_tiles = {}
    esums = {}

    def issue_scores(slabs, expS):
        sc_t = psum.tile([P, 1536], F32, tag="sc", name=uid("sc"))
        for i, (lc, hp, st, a) in enumerate(slabs):
            pa = 64 * a
            nc.tensor.matmul(
                sc_t[:, 512 * i : 512 * (i + 1)],
                kT[pa : pa + 64, hp, st * P : (st + 1) * P],
                qT[pa : pa + 64, hp, lc, :],
                start=True,
                stop=True,
            )
        with nc.allow_low_precision(reason="bf16 probs keep ~8 mantissa "
                                    "bits; tolerance 2e-2"):
            nc.scalar.activation(
                expS[:, 0 : len(slabs), :],
                sc_t[:, 0 : 512 * len(slabs)].rearrange(
                    "p (t l) -> p t l", l=LC
                ),
                mybir.ActivationFunctionType.Exp,
            )

    def issue_av(slabs, expS):
        for i, (lc, hp, st, a) in enumerate(slabs):
            if st == 0 and a == 0:
                av_tiles[(lc, hp)] = psumav.tile(
                    [P, 512], F32, tag="av", name=uid("av")
                )
                esums[(lc, hp)] = [
                    esump.tile([P, 512], F32, tag=f"es{j}", name=uid("es"))
                    for j in range(2)
                ]
            av = av_tiles[(lc, hp)]
            h = 2 * hp + a
            # col-tiled AV: head a -> PSUM partitions [64a, 64a+64).
            # start=True per head's first MM: the has_written clear applies
            # to the addressed region only (the baseline's two same-bank
            # broadcast MMs with start=True relied on exactly this).
            nc.tensor.matmul(
                av[64 * a : 64 * a + 64, :],
                v_sb[:, st, h, :],
                expS[:, i, :],
                start=(st == 0),
                stop=(st == 7),
                skip_group_check=True,
            )
            # denominator partial: esum_a += slab (fp32 accum on DVE)
            es = esums[(lc, hp)][a]
            if st == 0:
                nc.vector.tensor_copy(es[:], expS[:, i, :])
            else:
                nc.vector.tensor_tensor(
                    es[:], es[:], expS[:, i, :], mybir.AluOpType.add
                )
            if st == 7 and a == 1:
                finalize_hp(lc, hp)

    def finalize_hp(lc, hp):
        if lc not in oTs:
            oTs[lc] = work.tile([P, 4, LC], BF16, tag="oT", name=uid("oT"))
        oT = oTs[lc]
        av = av_tiles.pop((lc, hp))
        es = esums.pop((lc, hp))
        # cast esums to bf16 (matmul moving operand), then reduce over the
        # 128 s-partitions AND broadcast to 64 rows per head in one
        # ones-stationary matmul; reciprocal; multiply into oT.
        pd = psum2.tile([P, 512], F32, tag="misc", name=uid("pd"))
        for a in range(2):
            esb = work.tile([P, LC], BF16, tag="esb", name=uid("esb"))
            with nc.allow_low_precision(reason="bf16 denominator; tol 2e-2"):
                nc.vector.tensor_copy(esb[:], es[a][:])
            nc.tensor.matmul(
                pd[64 * a : 64 * a + 64, :], ones128[:], esb[:],
                start=True, stop=True,
            )
        bsb = work.tile([P, LC], BF16, tag="bsb", name=uid("bsb"))
        with nc.allow_low_precision(reason="denominator reciprocal in "
                                    "bf16; tolerance 2e-2"):
            nc.vector.reciprocal(bsb[:], pd[:])
            for a in range(2):
                nc.vector.tensor_mul(
                    oT[64 * a : 64 * a + 64, hp, :],
                    av[64 * a : 64 * a + 64, :],
                    bsb[64 * a : 64 * a + 64, :],
                )

    def outproj_group(lc, mt8):
        oT = oTs[lc]
        og = psum2.tile([P, 512], F32, tag="misc", name=uid("og"))
        for kt4 in range(4):
            nc.tensor.matmul(
                og[:],
                wo_sb[:, kt4, mt8 * P : (mt8 + 1) * P],
                oT[:, kt4, :],
                start=(kt4 == 0),
                stop=(kt4 == 3),
            )
        stg = work.tile([P, LC], BF16, tag="stg", name=uid("stg"))
        with nc.allow_low_precision(reason="bf16 partial output; host sums "
                                    "in fp32; tolerance 2e-2"):
            nc.vector.tensor_copy(stg[:], og[:])
        nc.sync.dma_start(
            out.ap()[mt8 * P : (mt8 + 1) * P, lc * LC : (lc + 1) * LC], stg[:]
        )
        if mt8 == 7:
            del oTs[lc]

    # slab stream: st-major within each (lc, hp) unit so each head's AV
    # accumulation sees st in order 0..7.
    slabs = [
        (lc, hp, st, a)
        for lc in range(N_LC)
        for hp in range(4)
        for st in range(8)
        for a in range(2)
    ]
    groups = [slabs[i : i + 3] for i in range(0, len(slabs), 3)]

    def fillers_for(slabs_done):
        # filler schedule keyed on lc-relative slab index (64 slabs per lc):
        # 4 Q-granules for lc+1 early in the lc, 8 out-proj groups for lc-1
        # spread across it (same cadence as the old per-batch schedule).
        for (lc, hp, st, a) in slabs_done:
            rel64 = hp * 16 + st * 2 + a
            if lc < N_LC - 1 and rel64 in (4, 12, 20, 28):
                yield ("q", lc + 1, (rel64 // 4 - 1) // 2)
            if lc > 0 and rel64 in (8, 16, 24, 32, 40, 48, 56, 60):
                yield ("o", lc - 1,
                       (8, 16, 24, 32, 40, 48, 56, 60).index(rel64))

    from collections import deque

    pend = deque()  # (slabs, expS), depth 2
    for gi, grp in enumerate(groups):
        expS = expp.tile([P, 3, LC], BF16, tag="expS", name=uid("e"))
        issue_scores(grp, expS)
        if len(pend) == 2:
            dslabs, dexp = pend.popleft()
            issue_av(dslabs, dexp)
            for kind, lc_, i_ in fillers_for(dslabs):
                if kind == "q":
                    q_granule(lc_, i_)
                else:
                    outproj_group(lc_, i_)
        pend.append((grp, expS))
    while pend:
        dslabs, dexp = pend.popleft()
        issue_av(dslabs, dexp)
        for kind, lc_, i_ in fillers_for(dslabs):
            if kind == "q":
                q_granule(lc_, i_)
            else:
                outproj_group(lc_, i_)
    for mt8 in range(8):
        outproj_group(N_LC - 1, mt8)


_NC_CACHE = {}


def _get_nc(reps=1, timing=False):
    if (reps, timing) not in _NC_CACHE:
        _NC_CACHE[(reps, timing)] = build_nc(reps, timing)
    return _NC_CACHE[(reps, timing)]


def make_in_maps(inputs):
    bf = ml_dtypes.bfloat16
    te = np.asarray(inputs["target_embedding"], np.float32)
    se = np.asarray(inputs["source_embedding"], np.float32)
    ve = np.asarray(inputs["value_embedding"], np.float32)
    Wq = np.asarray(inputs["Wq"], np.float32)
    Wk = np.asarray(inputs["Wk"], np.float32)
    Wv = np.asarray(inputs["Wv"], np.float32)
    Wo = np.asarray(inputs["Wo"], np.float32)
    bqv = np.asarray(inputs["bq"], np.float32)
    in_maps = []
    for core in range(8):
        b, g = divmod(core, 2)
        sl = slice(MQ * g, MQ * (g + 1))
        in_maps.append(
            {
                "xt": np.ascontiguousarray(te[b].T).astype(bf),
                "xs": np.ascontiguousarray(se[b].T).astype(bf),
                "xv": np.ascontiguousarray(ve[b].T).astype(bf),
                "wq": np.ascontiguousarray(Wq[:, sl]).astype(bf),
                "wk": np.ascontiguousarray(Wk[:, sl]).astype(bf),
                "wv": np.ascontiguousarray(Wv[:, sl]).astype(bf),
                "wo": np.ascontiguousarray(Wo[sl, :]).astype(bf),
                "bq": np.ascontiguousarray((bqv[sl] * SCALE).reshape(4, P).T),
            }
        )
    return in_maps


def assemble_output(results, inputs):
    bv = np.asarray(inputs["bv"], np.float32)
    bo = np.asarray(inputs["bo"], np.float32)
    Wo = np.asarray(inputs["Wo"], np.float32)
    corr = (bv @ Wo + bo).astype(np.float32)  # [D]
    out = np.empty((4, L, D), np.float32)
    for b in range(4):
        acc = results[2 * b]["out"].astype(np.float32) + results[
            2 * b + 1
        ]["out"].astype(np.float32)  # [D, L]
        out[b] = acc.T + corr
    return out


def _run_once(nc, in_maps, inputs):
    last_err = None
    for _attempt in range(3):
        try:
            res = run_bass_kernel_spmd(nc, in_maps, core_ids=list(range(8)))
            return assemble_output(res.results, inputs)
        except Exception as e:  # transient NRT device errors: retry
            last_err = e
    raise last_err


def kernel(**inputs) -> np.ndarray:
    nc = _get_nc(1)
    in_maps = make_in_maps(inputs)
    # Run twice and require agreement: guards against rare transient silent
    # device corruption (observed once after an abnormal device state).
    outs = [_run_once(nc, in_maps, inputs) for _ in range(2)]
    for _extra in range(2):
        scale = float(np.abs(outs[-1]).mean()) + 1e-30
        if any(
            np.abs(o - outs[-1]).max() < 1e-3 * scale for o in outs[:-1]
        ):
            return outs[-1]
        outs.append(_run_once(nc, in_maps, inputs))
    return outs[-1]


# revision 22
# speedup vs baseline: 1.1713x; 1.0586x over previous
"""Trainium2 Bass kernel for nn_AttentionLayer_13383118095164.

Cross-attention layer: q = target @ Wq + bq; k/v = source/value @ Wk/Wv + bk/bv;
out = softmax(q k^T / 8) v @ Wo + bo.   B=4, L=2048, S=1024, D=1024, H=16, E=64.

Sharding (8 cores): core c = (batch b = c//2, head-group g = c%2 of 8 heads).
Megatron-style: Q/K/V column-split by head group, Wo row-split; the two
head-group partial outputs per batch are summed on the host.

v3 design notes (baseline 475us this session, v2 449us, v3 ~434us; HW
microbenches drove each rev):
  * DMA: a single HWDGE queue sustains only ~148GB/s (measured); the 30MB
    of input traffic is split across the two HWDGE queues (SP +
    Activation), which run in parallel at ~148GB/s each.
  * ACT: exp costs ~1.3us per [128,2,512] PSUM-sourced instruction but only
    ~1.65us for a 4-bank [128,4,512] one (per-inst overhead dominates), so
    scores per batch land in ONE [P,2048] PSUM tile (4 banks) consumed by
    ONE exp.  PSUM: sc(4) + av(2) + misc(2) banks.
  * Scores are issued FIRST in each batch's PE program (they gate the next
    exp; AV + fillers follow) and the two 64-row PE tiles (head pair) are
    interleaved so they run concurrently (64x128 row tiling, measured
    118ns/MM vs 150 sequential).
  * Q-projection copyback moved from ACT (the attention pacer) to DVE
    (tensor_scalar mul+add).
  * out written bf16 (halves output DMA); host sums partials in fp32.
  * Timing loop unrolled 4x per For_i iteration (each hardware-loop
    boundary costs ~2.9us of drain, measured).
  Rejected after measuring slower: DVE evacuation of scores to SBUF for a
  cheaper SBUF-sourced exp (DVE queueing delayed exp more than the PSUM
  read penalty it saved: 490us vs 434us).
  Device layout otherwise as v1/v2: everything transposed, softmax
  denominator via a ones-column in V (M=65 AV matmuls), bk dropped
  (softmax shift invariance), bv@Wo+bo folded into a host constant.
"""

import numpy as np
import ml_dtypes

import concourse.bass as bass
import concourse.tile as tile
from concourse import mybir
from concourse.bass_utils import run_bass_kernel_spmd

P = 128
D = 1024  # d_model
DL = 4096  # d_llm
L = 2048  # target length
S = 1024  # source length
MQ = 512  # per-core q/k/v dims (8 heads x 64)
E = 64
E1 = E + 1
HG = 8  # heads per core
LC = 512  # l-chunk
N_LC = L // LC
SCALE = 0.125  # 1/sqrt(E)

BF16 = mybir.dt.bfloat16
F32 = mybir.dt.float32


def _split_multi_waits(nc):
    """This walrus build rejects >1 sync wait per instruction: split extras
    onto single-wait NOPs on the same engine immediately before (same program
    order on the same queue => identical semantics)."""
    for f in nc.m.functions:
        for blk in f.blocks:
            new_insts = []
            for inst in blk.instructions:
                si = inst.sync_info
                if si is not None and si.on_wait and len(si.on_wait) > 1:
                    waits = list(si.on_wait)
                    for w in waits[:-1]:
                        nop = mybir.InstNoOp(
                            name=f"I-waitsplit-{nc.next_id()}", ins=[], outs=[]
                        )
                        nop.engine = inst.engine
                        nop.sync_info = mybir.SyncInfo(on_wait=[w], on_update=[])
                        new_insts.append(nop)
                    si.on_wait = [waits[-1]]
                new_insts.append(inst)
            blk.instructions[:] = new_insts


def build_nc(reps: int = 1, timing: bool = False, phases: str = "all"):
    """timing=True: identical device program, but all real tensors are
    Internal DRAM (uninitialized) with tiny dummy external I/O, so timing
    runs ship no host data and the slope measurement is low-variance. The
    engines here are data-independent in speed, so per-rep time is the same."""
    from contextlib import ExitStack

    nc = bass.Bass(trn_type="TRN2", target_bir_lowering=False, debug=False)

    ikind = "Internal" if timing else "ExternalInput"
    okind = "Internal" if timing else "ExternalOutput"
    xt = nc.dram_tensor("xt", [D, L], BF16, kind=ikind)  # X_t^T
    xs = nc.dram_tensor("xs", [DL, S], BF16, kind=ikind)  # X_s^T
    xv = nc.dram_tensor("xv", [DL, S], BF16, kind=ikind)  # X_v^T
    wq = nc.dram_tensor("wq", [D, MQ], BF16, kind=ikind)
    wk = nc.dram_tensor("wk", [DL, MQ], BF16, kind=ikind)
    wv = nc.dram_tensor("wv", [DL, MQ], BF16, kind=ikind)
    wo = nc.dram_tensor("wo", [MQ, D], BF16, kind=ikind)
    bq = nc.dram_tensor("bq", [P, 4], F32, kind=ikind)  # bq/8 as [p, mt]
    out = nc.dram_tensor("out", [D, L], BF16, kind=okind)  # out^T partial
    if timing:
        dummy = nc.dram_tensor("tdin", [1, 4], F32, kind="ExternalInput")
        dumout = nc.dram_tensor("tdout", [1, 4], F32, kind="ExternalOutput")

    with tile.TileContext(nc) as tc, ExitStack() as ctx:
        const = ctx.enter_context(tc.tile_pool(name="const", bufs=1))
        resident = ctx.enter_context(tc.tile_pool(name="resident", bufs=1))
        stream = ctx.enter_context(tc.tile_pool(name="stream", bufs=6))
        stream2 = ctx.enter_context(tc.tile_pool(name="stream2", bufs=6))
        psum = ctx.enter_context(tc.tile_pool(name="psum", bufs=2, space="PSUM"))
        psumav = ctx.enter_context(tc.tile_pool(name="psumav", bufs=2, space="PSUM"))
        psum2 = ctx.enter_context(tc.tile_pool(name="psum2", bufs=2, space="PSUM"))

        # ---- constants ----
        if timing:
            dtile = const.tile([1, 4], F32, name="dtile")
            nc.sync.dma_start(dtile[:], dummy.ap())
            nc.sync.dma_start(dumout.ap(), dtile[:])
        bq_sb = const.tile([P, 4], F32, name="bq_sb")
        nc.sync.dma_start(bq_sb[:], bq.ap())
        ones128 = const.tile([P, E], BF16, name="ones128")
        nc.vector.memset(ones128[:], 1.0)

        # ---- residents ----
        kT = resident.tile([P, 4, S], BF16, name="kT")  # [p, mt, s]
        v_sb = resident.tile([P, 8, HG, E], BF16, name="v_sb")  # [p, st, h, e]
        esump = ctx.enter_context(tc.tile_pool(name="esump", bufs=3))
        expp = ctx.enter_context(tc.tile_pool(name="expp", bufs=4))
        qT = resident.tile([P, 4, N_LC, LC], BF16, name="qT")  # [p, mt, lc, l]
        xt_sb = resident.tile([P, D // P, L], BF16, name="xt_sb")  # [p, kt, l]
        wq_sb = resident.tile([P, D // P, MQ], BF16, name="wq_sb")
        wo_sb = resident.tile([P, MQ // P, D], BF16, name="wo_sb")

        def _body(work):
            _emit_rep(
                nc, psum, psumav, psum2, esump, expp, work, stream, stream2,
                xt, xs, xv, wq, wk, wv, wo,
                bq_sb, ones128, kT, v_sb, qT, xt_sb, wq_sb, wo_sb, out,
                phases,
            )

        if reps == 1:
            with tc.tile_pool(name="work", bufs=4) as work:
                _body(work)
        else:
            # Unroll reps per For_i iteration: hardware-loop iterations
            # cannot overlap, but Tile schedules across the unrolled bodies,
            # so rep boundaries inside an iteration pipeline.
            unroll = 4 if reps % 4 == 0 else 2
            assert reps % 2 == 0 or reps == 1, "reps must be even"
            with tc.For_i(0, reps // unroll, 1):
                with tc.tile_pool(name="work", bufs=4) as work:
                    for _ in range(unroll):
                        _body(work)

    _split_multi_waits(nc)
    return nc


def _emit_rep(nc, psum, psumav, psum2, esump, expp, work, stream, stream2,
              xt, xs, xv, wq, wk, wv, wo,
              bq_sb, ones128, kT, v_sb, qT, xt_sb, wq_sb, wo_sb, out,
              phases="all"):
    names = [0]
    do_ab = phases in ("all", "ab")
    do_attn = phases in ("all", "attn")

    def uid(s):
        names[0] += 1
        return f"{s}_{names[0]}"

    def alloc8(pfx):
        """All 8 PSUM banks as 8 [P,512] accumulators (2 sc + av + misc)."""
        b0 = psum.tile([P, 1024], F32, tag="sc", name=uid(f"{pfx}b0"))
        b1 = psum.tile([P, 1024], F32, tag="sc", name=uid(f"{pfx}b1"))
        a0 = psumav.tile([P, 512], F32, tag="av", name=uid(f"{pfx}a0"))
        a1 = psumav.tile([P, 512], F32, tag="av", name=uid(f"{pfx}a1"))
        m0 = psum2.tile([P, 512], F32, tag="misc", name=uid(f"{pfx}m0"))
        m1 = psum2.tile([P, 512], F32, tag="misc", name=uid(f"{pfx}m1"))
        return [
            b0[:, 0:512], b0[:, 512:1024],
            b1[:, 0:512], b1[:, 512:1024],
            a0[:], a1[:],
            m0[:], m1[:],
        ]

    def q_granule(lc, mt):
        qm = psum2.tile([P, 512], F32, tag="misc", name=uid("qm"))
        for kt in range(D // P):
            nc.tensor.matmul(
                qm[:],
                wq_sb[:, kt, mt * P : (mt + 1) * P],
                xt_sb[:, kt, lc * LC : (lc + 1) * LC],
                start=(kt == 0),
                stop=(kt == D // P - 1),
            )
        with nc.allow_low_precision(reason="bf16 q keeps ~8 mantissa bits"):
            nc.vector.tensor_scalar(
                qT[:, mt, lc, :],
                qm[:],
                SCALE,
                bq_sb[:, mt : mt + 1],
                mybir.AluOpType.mult,
                mybir.AluOpType.add,
            )

    # ---------- Phase A: K^T = Wk_g^T @ X_s^T -> kT[p, mt, s] ----------
    # wk + residents ride the SP queue; xs streams on the ACT HWDGE queue
    # (idle during projections) so the two queues halve the DMA wall time.
    if do_ab:
        kacc = alloc8("k")
        for kt in range(DL // P):
            wk_t = stream.tile([P, MQ], BF16, tag="wk_t", name=uid("wk_t"))
            nc.sync.dma_start(wk_t[:], wk.ap()[kt * P : (kt + 1) * P, :])
            xs_t = stream2.tile([P, S], BF16, tag="xs_t", name=uid("xs_t"))
            nc.scalar.dma_start(xs_t[:], xs.ap()[kt * P : (kt + 1) * P, :])
            if kt >= 2 and kt % 3 == 2 and (kt - 2) // 3 < 8:
                j = (kt - 2) // 3
                nc.sync.dma_start(xt_sb[:, j, :], xt.ap()[j * P : (j + 1) * P, :])
            elif kt == 27:
                nc.sync.dma_start(
                    wq_sb[:], wq.ap().rearrange("(kt p) m -> p kt m", p=P)
                )
            elif kt == 30:
                nc.sync.dma_start(
                    wo_sb[:], wo.ap().rearrange("(kt p) d -> p kt d", p=P)
                )
            for mt in range(4):
                for sc in range(2):
                    nc.tensor.matmul(
                        kacc[mt * 2 + sc],
                        wk_t[:, mt * P : (mt + 1) * P],
                        xs_t[:, sc * 512 : (sc + 1) * 512],
                        start=(kt == 0),
                        stop=(kt == DL // P - 1),
                    )
        with nc.allow_low_precision(reason="bf16 operands keep ~8 mantissa "
                                    "bits; rel tolerance is 2e-2"):
            for i in range(8):
                nc.vector.tensor_copy(
                    kT[:, i // 2, (i % 2) * 512 : (i % 2 + 1) * 512], kacc[i]
                )

        # ---------- Phase Q0: Q proj for lc=0 ----------
        for mt in range(4):
            q_granule(0, mt)

        # ---------- Phase B: V = X_v @ Wv_g -> v_sb[p, st, h, 0:64] --------
        vacc = alloc8("v")
        for kt in range(DL // P):
            wv_t = stream.tile([P, MQ], BF16, tag="wk_t", name=uid("wv_t"))
            nc.sync.dma_start(wv_t[:], wv.ap()[kt * P : (kt + 1) * P, :])
            xv_t = stream2.tile([P, S], BF16, tag="xs_t", name=uid("xv_t"))
            # alternate xv between the two HWDGE queues to balance load
            xv_q = nc.scalar if kt % 2 == 0 else nc.sync
            xv_q.dma_start(xv_t[:], xv.ap()[kt * P : (kt + 1) * P, :])
            for st in range(8):
                nc.tensor.matmul(
                    vacc[st],
                    xv_t[:, st * P : (st + 1) * P],
                    wv_t[:, :],
                    start=(kt == 0),
                    stop=(kt == DL // P - 1),
                )
        with nc.allow_low_precision(reason="bf16 v keeps ~8 mantissa bits"):
            for st in range(8):
                nc.vector.tensor_copy(
                    v_sb[:, st, :, 0:E],
                    vacc[st].rearrange("p (h e) -> p h e", e=E),
                )

    if not do_ab:
        # attention-only ablation: init residents so Tile sees writers
        # (memsets distribute across engines; ~upper-bounds attention span)
        for t_ in (kT, v_sb, qT, xt_sb, wq_sb, wo_sb):
            nc.vector.memset(t_[:], 0.01)

    if not do_attn:
        return

    # ---------- Phase C: attention, slab pipeline ----------
    # Unit of work = one score slab (lc, hp, st, a): [128 s, 512 l] for one
    # head-half.  Slabs stream st-major within each (lc, hp) unit; groups of
    # 3 consecutive slabs share one [P,1536] sc tile (3 banks) and ONE exp.
    # sc rotates x2 (6 banks) so exp(k+1) never waits on bank recycling:
    # ACT runs back-to-back.  AV is col-tiled (M=64, heads stacked on PSUM
    # partitions) into a 1-bank av tile; softmax denominators come from DVE
    # slab-sums (fp32) reduced+broadcast by a ones-stationary matmul.
    oTs = {}
    av_tiles = {}
    esums = {}

    def issue_scores(slabs, expS):
        sc_t = psum.tile([P, 1024], F32, tag="sc", name=uid("sc"))
        for i, (lc, hp, st, a) in enumerate(slabs):
            pa = 64 * a
            nc.tensor.matmul(
                sc_t[:, 512 * i : 512 * (i + 1)],
                kT[pa : pa + 64, hp, st * P : (st + 1) * P],
                qT[pa : pa + 64, hp, lc, :],
                start=True,
                stop=True,
            )
        with nc.allow_low_precision(reason="bf16 probs keep ~8 mantissa "
                                    "bits; tolerance 2e-2"):
            nc.scalar.activation(
                expS[:, 0 : len(slabs), :],
                sc_t[:, 0 : 512 * len(slabs)].rearrange(
                    "p (t l) -> p t l", l=LC
                ),
                mybir.ActivationFunctionType.Exp,
            )

    def issue_av(slabs, expS):
        for i, (lc, hp, st, a) in enumerate(slabs):
            if st == 0 and a == 0:
                av_tiles[(lc, hp)] = psumav.tile(
                    [P, 512], F32, tag="av", name=uid("av")
                )
                esums[(lc, hp)] = [
                    esump.tile([P, 512], F32, tag=f"es{j}", name=uid("es"))
                    for j in range(2)
                ]
            av = av_tiles[(lc, hp)]
            h = 2 * hp + a
            # col-tiled AV: head a -> PSUM partitions [64a, 64a+64).
            # start=True per head's first MM: the has_written clear applies
            # to the addressed region only (the baseline's two same-bank
            # broadcast MMs with start=True relied on exactly this).
            nc.tensor.matmul(
                av[64 * a : 64 * a + 64, :],
                v_sb[:, st, h, :],
                expS[:, i, :],
                start=(st == 0),
                stop=(st == 7),
                skip_group_check=True,
            )
            # denominator partial: esum_a += slab (fp32 accum on DVE)
            es = esums[(lc, hp)][a]
            if st == 0:
                nc.vector.tensor_copy(es[:], expS[:, i, :])
            else:
                nc.vector.tensor_tensor(
                    es[:], es[:], expS[:, i, :], mybir.AluOpType.add
                )
            if st == 7 and a == 1:
                finalize_hp(lc, hp)

    def finalize_hp(lc, hp):
        if lc not in oTs:
            oTs[lc] = work.tile([P, 4, LC], BF16, tag="oT", name=uid("oT"))
        oT = oTs[lc]
        av = av_tiles.pop((lc, hp))
        es = esums.pop((lc, hp))
        # cast esums to bf16 (matmul moving operand), then reduce over the
        # 128 s-partitions AND broadcast to 64 rows per head in one
        # ones-stationary matmul; reciprocal; multiply into oT.
        pd = psum2.tile([P, 512], F32, tag="misc", name=uid("pd"))
        for a in range(2):
            esb = work.tile([P, LC], BF16, tag="esb", name=uid("esb"))
            with nc.allow_low_precision(reason="bf16 denominator; tol 2e-2"):
                nc.vector.tensor_copy(esb[:], es[a][:])
            nc.tensor.matmul(
                pd[64 * a : 64 * a + 64, :], ones128[:], esb[:],
                start=True, stop=True,
            )
        bsb = work.tile([P, LC], BF16, tag="bsb", name=uid("bsb"))
        with nc.allow_low_precision(reason="denominator reciprocal in "
                                    "bf16; tolerance 2e-2"):
            nc.vector.reciprocal(bsb[:], pd[:])
            for a in range(2):
                nc.vector.tensor_mul(
                    oT[64 * a : 64 * a + 64, hp, :],
                    av[64 * a : 64 * a + 64, :],
                    bsb[64 * a : 64 * a + 64, :],
                )

    def outproj_group(lc, mt8):
        oT = oTs[lc]
        og = psum2.tile([P, 512], F32, tag="misc", name=uid("og"))
        for kt4 in range(4):
            nc.tensor.matmul(
                og[:],
                wo_sb[:, kt4, mt8 * P : (mt8 + 1) * P],
                oT[:, kt4, :],
                start=(kt4 == 0),
                stop=(kt4 == 3),
            )
        stg = work.tile([P, LC], BF16, tag="stg", name=uid("stg"))
        with nc.allow_low_precision(reason="bf16 partial output; host sums "
                                    "in fp32; tolerance 2e-2"):
            nc.vector.tensor_copy(stg[:], og[:])
        nc.sync.dma_start(
            out.ap()[mt8 * P : (mt8 + 1) * P, lc * LC : (lc + 1) * LC], stg[:]
        )
        if mt8 == 7:
            del oTs[lc]

    # slab stream: st-major within each (lc, hp) unit so each head's AV
    # accumulation sees st in order 0..7.
    slabs = [
        (lc, hp, st, a)
        for lc in range(N_LC)
        for hp in range(4)
        for st in range(8)
        for a in range(2)
    ]
    groups = [slabs[i : i + 2] for i in range(0, len(slabs), 2)]

    def fillers_for(slabs_done):
        # filler schedule keyed on lc-relative slab index (64 slabs per lc):
        # 4 Q-granules for lc+1 early in the lc, 8 out-proj groups for lc-1
        # spread across it (same cadence as the old per-batch schedule).
        for (lc, hp, st, a) in slabs_done:
            rel64 = hp * 16 + st * 2 + a
            if lc < N_LC - 1 and rel64 in (4, 12, 20, 28):
                yield ("q", lc + 1, (rel64 // 4 - 1) // 2)
            if lc > 0 and rel64 in (8, 16, 24, 32, 40, 48, 56, 60):
                yield ("o", lc - 1,
                       (8, 16, 24, 32, 40, 48, 56, 60).index(rel64))

    from collections import deque

    pend = deque()  # (slabs, expS), depth 2
    for gi, grp in enumerate(groups):
        expS = expp.tile([P, 2, LC], BF16, tag="expS", name=uid("e"))
        issue_scores(grp, expS)
        if len(pend) == 2:
            dslabs, dexp = pend.popleft()
            issue_av(dslabs, dexp)
            for kind, lc_, i_ in fillers_for(dslabs):
                if kind == "q":
                    q_granule(lc_, i_)
                else:
                    outproj_group(lc_, i_)
        pend.append((grp, expS))
    while pend:
        dslabs, dexp = pend.popleft()
        issue_av(dslabs, dexp)
        for kind, lc_, i_ in fillers_for(dslabs):
            if kind == "q":
                q_granule(lc_, i_)
            else:
                outproj_group(lc_, i_)
    for mt8 in range(8):
        outproj_group(N_LC - 1, mt8)


_NC_CACHE = {}


def _get_nc(reps=1, timing=False):
    if (reps, timing) not in _NC_CACHE:
        _NC_CACHE[(reps, timing)] = build_nc(reps, timing)
    return _NC_CACHE[(reps, timing)]


def make_in_maps(inputs):
    bf = ml_dtypes.bfloat16
    te = np.asarray(inputs["target_embedding"], np.float32)
    se = np.asarray(inputs["source_embedding"], np.float32)
    ve = np.asarray(inputs["value_embedding"], np.float32)
    Wq = np.asarray(inputs["Wq"], np.float32)
    Wk = np.asarray(inputs["Wk"], np.float32)
    Wv = np.asarray(inputs["Wv"], np.float32)
    Wo = np.asarray(inputs["Wo"], np.float32)
    bqv = np.asarray(inputs["bq"], np.float32)
    in_maps = []
    for core in range(8):
        b, g = divmod(core, 2)
        sl = slice(MQ * g, MQ * (g + 1))
        in_maps.append(
            {
                "xt": np.ascontiguousarray(te[b].T).astype(bf),
                "xs": np.ascontiguousarray(se[b].T).astype(bf),
                "xv": np.ascontiguousarray(ve[b].T).astype(bf),
                "wq": np.ascontiguousarray(Wq[:, sl]).astype(bf),
                "wk": np.ascontiguousarray(Wk[:, sl]).astype(bf),
                "wv": np.ascontiguousarray(Wv[:, sl]).astype(bf),
                "wo": np.ascontiguousarray(Wo[sl, :]).astype(bf),
                "bq": np.ascontiguousarray((bqv[sl] * SCALE).reshape(4, P).T),
            }
        )
    return in_maps


def assemble_output(results, inputs):
    bv = np.asarray(inputs["bv"], np.float32)
    bo = np.asarray(inputs["bo"], np.float32)
    Wo = np.asarray(inputs["Wo"], np.float32)
    corr = (bv @ Wo + bo).astype(np.float32)  # [D]
    out = np.empty((4, L, D), np.float32)
    for b in range(4):
        acc = results[2 * b]["out"].astype(np.float32) + results[
            2 * b + 1
        ]["out"].astype(np.float32)  # [D, L]
        out[b] = acc.T + corr
    return out


def _run_once(nc, in_maps, inputs):
    last_err = None
    for _attempt in range(3):
        try:
            res = run_bass_kernel_spmd(nc, in_maps, core_ids=list(range(8)))
            return assemble_output(res.results, inputs)
        except Exception as e:  # transient NRT device errors: retry
            last_err = e
    raise last_err


def kernel(**inputs) -> np.ndarray:
    nc = _get_nc(1)
    in_maps = make_in_maps(inputs)
    # Run twice and require agreement: guards against rare transient silent
    # device corruption (observed once after an abnormal device state).
    outs = [_run_once(nc, in_maps, inputs) for _ in range(2)]
    for _extra in range(2):
        scale = float(np.abs(outs[-1]).mean()) + 1e-30
        if any(
            np.abs(o - outs[-1]).max() < 1e-3 * scale for o in outs[:-1]
        ):
            return outs[-1]
        outs.append(_run_once(nc, in_maps, inputs))
    return outs[-1]


# revision 25
# speedup vs baseline: 1.2552x; 1.0717x over previous
"""Trainium2 Bass kernel for nn_AttentionLayer_13383118095164.

Cross-attention layer: q = target @ Wq + bq; k/v = source/value @ Wk/Wv + bk/bv;
out = softmax(q k^T / 8) v @ Wo + bo.   B=4, L=2048, S=1024, D=1024, H=16, E=64.

Sharding (8 cores): core c = (batch b = c//2, head-group g = c%2 of 8 heads).
Megatron-style: Q/K/V column-split by head group, Wo row-split; the two
head-group partial outputs per batch are summed on the host.

v6 design notes (session journey: 476us baseline -> 449 -> 434/424 (v3)
-> 406us; every rev driven by HW microbenches / phase ablation):
  * DMA: one HWDGE queue sustains only ~148GB/s (measured); the ~30MB of
    input traffic is split across the two parallel HWDGE queues (SP +
    Activation).  Projection phases ablate to ~115us/rep (DMA-bound).
  * Attention is a slab pipeline: slab = (lc, hp, st, head-half a) score
    matmul [128s x 512l]; pairs of slabs share one 2-bank sc tile and ONE
    exp (rotation x2 so exp never waits on bank recycling).  Scores go
    FIRST in each group's PE program; AV trails 2 groups; Q-proj/out-proj
    fillers ride per-lc slots.
  * AV is col-tiled (M=64, heads stacked on PSUM partitions, 1 bank per
    (lc,hp)); softmax denominators: DVE accumulates fp32 slab sums, a
    ones-stationary matmul reduces over s AND broadcasts per head, then
    reciprocal + multiply.  (The v1-v3 ones-column-in-V / M=65 trick was
    dropped to free the bank.)
  * PSUM: sc 2x2 + av 2x1 + misc 2x1 = 8 banks.  Phase ablation showed
    attention is latency-chain bound (306us for ~125us of ACT work with
    single-buffered av/misc): double-buffering av and misc removed the
    unit-boundary and filler-chain PE stalls and measured faster than
    bigger-but-single-buffered exp tiles (406 vs 424-434).
  * start=True on a matmul clears has_written only for the ADDRESSED
    region, not the whole bank (two same-bank partition-half accumulation
    groups each carry their own start=True; verified on HW).
  * Q-proj copyback on DVE (tensor_scalar); out written bf16; timing loop
    unrolled 4x per For_i iteration (boundary costs ~2.9us).
  Rejected after measuring slower: DVE evacuation of scores to SBUF for
  cheaper SBUF-sourced exp (490us); deeper stream/work pool bufs (470us);
  3-slab sc tiles with single-buffered av/misc (430us).
  Device layout otherwise as v1: everything transposed, bk dropped
  (softmax shift invariance), bv@Wo+bo folded into a host constant.
"""

import numpy as np
import ml_dtypes

import concourse.bass as bass
import concourse.tile as tile
from concourse import mybir
from concourse.bass_utils import run_bass_kernel_spmd

P = 128
D = 1024  # d_model
DL = 4096  # d_llm
L = 2048  # target length
S = 1024  # source length
MQ = 512  # per-core q/k/v dims (8 heads x 64)
E = 64
E1 = E + 1
HG = 8  # heads per core
LC = 512  # l-chunk
N_LC = L // LC
SCALE = 0.125  # 1/sqrt(E)

BF16 = mybir.dt.bfloat16
F32 = mybir.dt.float32


def _split_multi_waits(nc):
    """This walrus build rejects >1 sync wait per instruction: split extras
    onto single-wait NOPs on the same engine immediately before (same program
    order on the same queue => identical semantics)."""
    for f in nc.m.functions:
        for blk in f.blocks:
            new_insts = []
            for inst in blk.instructions:
                si = inst.sync_info
                if si is not None and si.on_wait and len(si.on_wait) > 1:
                    waits = list(si.on_wait)
                    for w in waits[:-1]:
                        nop = mybir.InstNoOp(
                            name=f"I-waitsplit-{nc.next_id()}", ins=[], outs=[]
                        )
                        nop.engine = inst.engine
                        nop.sync_info = mybir.SyncInfo(on_wait=[w], on_update=[])
                        new_insts.append(nop)
                    si.on_wait = [waits[-1]]
                new_insts.append(inst)
            blk.instructions[:] = new_insts


def build_nc(reps: int = 1, timing: bool = False, phases: str = "all"):
    """timing=True: identical device program, but all real tensors are
    Internal DRAM (uninitialized) with tiny dummy external I/O, so timing
    runs ship no host data and the slope measurement is low-variance. The
    engines here are data-independent in speed, so per-rep time is the same."""
    from contextlib import ExitStack

    nc = bass.Bass(trn_type="TRN2", target_bir_lowering=False, debug=False)

    ikind = "Internal" if timing else "ExternalInput"
    okind = "Internal" if timing else "ExternalOutput"
    xt = nc.dram_tensor("xt", [D, L], BF16, kind=ikind)  # X_t^T
    xs = nc.dram_tensor("xs", [DL, S], BF16, kind=ikind)  # X_s^T
    xv = nc.dram_tensor("xv", [DL, S], BF16, kind=ikind)  # X_v^T
    wq = nc.dram_tensor("wq", [D, MQ], BF16, kind=ikind)
    wk = nc.dram_tensor("wk", [DL, MQ], BF16, kind=ikind)
    wv = nc.dram_tensor("wv", [DL, MQ], BF16, kind=ikind)
    wo = nc.dram_tensor("wo", [MQ, D], BF16, kind=ikind)
    bq = nc.dram_tensor("bq", [P, 4], F32, kind=ikind)  # bq/8 as [p, mt]
    out = nc.dram_tensor("out", [D, L], BF16, kind=okind)  # out^T partial
    if timing:
        dummy = nc.dram_tensor("tdin", [1, 4], F32, kind="ExternalInput")
        dumout = nc.dram_tensor("tdout", [1, 4], F32, kind="ExternalOutput")

    with tile.TileContext(nc) as tc, ExitStack() as ctx:
        const = ctx.enter_context(tc.tile_pool(name="const", bufs=1))
        resident = ctx.enter_context(tc.tile_pool(name="resident", bufs=1))
        stream = ctx.enter_context(tc.tile_pool(name="stream", bufs=6))
        stream2 = ctx.enter_context(tc.tile_pool(name="stream2", bufs=6))
        psum = ctx.enter_context(tc.tile_pool(name="psum", bufs=2, space="PSUM"))
        psumav = ctx.enter_context(tc.tile_pool(name="psumav", bufs=2, space="PSUM"))
        psum2 = ctx.enter_context(tc.tile_pool(name="psum2", bufs=2, space="PSUM"))

        # ---- constants ----
        if timing:
            dtile = const.tile([1, 4], F32, name="dtile")
            nc.sync.dma_start(dtile[:], dummy.ap())
            nc.sync.dma_start(dumout.ap(), dtile[:])
        bq_sb = const.tile([P, 4], F32, name="bq_sb")
        nc.sync.dma_start(bq_sb[:], bq.ap())
        ones128 = const.tile([P, E], BF16, name="ones128")
        nc.vector.memset(ones128[:], 1.0)

        # ---- residents ----
        kT = resident.tile([P, 4, S], BF16, name="kT")  # [p, mt, s]
        v_sb = resident.tile([P, 8, HG, E], BF16, name="v_sb")  # [p, st, h, e]
        esump = ctx.enter_context(tc.tile_pool(name="esump", bufs=3))
        expp = ctx.enter_context(tc.tile_pool(name="expp", bufs=4))
        qT = resident.tile([P, 4, N_LC, LC], BF16, name="qT")  # [p, mt, lc, l]
        xt_sb = resident.tile([P, D // P, L], BF16, name="xt_sb")  # [p, kt, l]
        wq_sb = resident.tile([P, D // P, MQ], BF16, name="wq_sb")
        wo_sb = resident.tile([P, MQ // P, D], BF16, name="wo_sb")

        def _body(work):
            _emit_rep(
                nc, psum, psumav, psum2, esump, expp, work, stream, stream2,
                xt, xs, xv, wq, wk, wv, wo,
                bq_sb, ones128, kT, v_sb, qT, xt_sb, wq_sb, wo_sb, out,
                phases,
            )

        if reps == 1:
            with tc.tile_pool(name="work", bufs=4) as work:
                _body(work)
        else:
            # Unroll reps per For_i iteration: hardware-loop iterations
            # cannot overlap, but Tile schedules across the unrolled bodies,
            # so rep boundaries inside an iteration pipeline.
            unroll = 4 if reps % 4 == 0 else 2
            assert reps % 2 == 0 or reps == 1, "reps must be even"
            with tc.For_i(0, reps // unroll, 1):
                with tc.tile_pool(name="work", bufs=4) as work:
                    for _ in range(unroll):
                        _body(work)

    _split_multi_waits(nc)
    return nc


def _emit_rep(nc, psum, psumav, psum2, esump, expp, work, stream, stream2,
              xt, xs, xv, wq, wk, wv, wo,
              bq_sb, ones128, kT, v_sb, qT, xt_sb, wq_sb, wo_sb, out,
              phases="all"):
    names = [0]
    do_ab = phases in ("all", "ab")
    do_attn = phases in ("all", "attn")

    def uid(s):
        names[0] += 1
        return f"{s}_{names[0]}"

    def alloc8(pfx):
        """All 8 PSUM banks as 8 [P,512] accumulators (2 sc + av + misc)."""
        b0 = psum.tile([P, 1024], F32, tag="sc", name=uid(f"{pfx}b0"))
        b1 = psum.tile([P, 1024], F32, tag="sc", name=uid(f"{pfx}b1"))
        a0 = psumav.tile([P, 512], F32, tag="av", name=uid(f"{pfx}a0"))
        a1 = psumav.tile([P, 512], F32, tag="av", name=uid(f"{pfx}a1"))
        m0 = psum2.tile([P, 512], F32, tag="misc", name=uid(f"{pfx}m0"))
        m1 = psum2.tile([P, 512], F32, tag="misc", name=uid(f"{pfx}m1"))
        return [
            b0[:, 0:512], b0[:, 512:1024],
            b1[:, 0:512], b1[:, 512:1024],
            a0[:], a1[:],
            m0[:], m1[:],
        ]

    def q_granule(lc, mt):
        qm = psum2.tile([P, 512], F32, tag="misc", name=uid("qm"))
        for kt in range(D // P):
            nc.tensor.matmul(
                qm[:],
                wq_sb[:, kt, mt * P : (mt + 1) * P],
                xt_sb[:, kt, lc * LC : (lc + 1) * LC],
                start=(kt == 0),
                stop=(kt == D // P - 1),
            )
        with nc.allow_low_precision(reason="bf16 q keeps ~8 mantissa bits"):
            nc.vector.tensor_scalar(
                qT[:, mt, lc, :],
                qm[:],
                SCALE,
                bq_sb[:, mt : mt + 1],
                mybir.AluOpType.mult,
                mybir.AluOpType.add,
            )

    # ---------- Phase A: K^T = Wk_g^T @ X_s^T -> kT[p, mt, s] ----------
    # wk + residents ride the SP queue; xs streams on the ACT HWDGE queue
    # (idle during projections) so the two queues halve the DMA wall time.
    if do_ab:
        kacc = alloc8("k")
        for kt in range(DL // P):
            wk_t = stream.tile([P, MQ], BF16, tag="wk_t", name=uid("wk_t"))
            nc.sync.dma_start(wk_t[:], wk.ap()[kt * P : (kt + 1) * P, :])
            xs_t = stream2.tile([P, S], BF16, tag="xs_t", name=uid("xs_t"))
            nc.scalar.dma_start(xs_t[:], xs.ap()[kt * P : (kt + 1) * P, :])
            if kt >= 2 and kt % 3 == 2 and (kt - 2) // 3 < 8:
                j = (kt - 2) // 3
                nc.sync.dma_start(xt_sb[:, j, :], xt.ap()[j * P : (j + 1) * P, :])
            elif kt == 27:
                nc.sync.dma_start(
                    wq_sb[:], wq.ap().rearrange("(kt p) m -> p kt m", p=P)
                )
            elif kt == 30:
                nc.sync.dma_start(
                    wo_sb[:], wo.ap().rearrange("(kt p) d -> p kt d", p=P)
                )
            for mt in range(4):
                for sc in range(2):
                    nc.tensor.matmul(
                        kacc[mt * 2 + sc],
                        wk_t[:, mt * P : (mt + 1) * P],
                        xs_t[:, sc * 512 : (sc + 1) * 512],
                        start=(kt == 0),
                        stop=(kt == DL // P - 1),
                    )
        with nc.allow_low_precision(reason="bf16 operands keep ~8 mantissa "
                                    "bits; rel tolerance is 2e-2"):
            for i in range(8):
                nc.vector.tensor_copy(
                    kT[:, i // 2, (i % 2) * 512 : (i % 2 + 1) * 512], kacc[i]
                )

        # ---------- Phase Q0: Q proj for lc=0 ----------
        for mt in range(4):
            q_granule(0, mt)

        # ---------- Phase B: V = X_v @ Wv_g -> v_sb[p, st, h, 0:64] --------
        vacc = alloc8("v")
        for kt in range(DL // P):
            wv_t = stream.tile([P, MQ], BF16, tag="wk_t", name=uid("wv_t"))
            nc.sync.dma_start(wv_t[:], wv.ap()[kt * P : (kt + 1) * P, :])
            xv_t = stream2.tile([P, S], BF16, tag="xs_t", name=uid("xv_t"))
            # alternate xv between the two HWDGE queues to balance load
            xv_q = nc.scalar if kt % 2 == 0 else nc.sync
            xv_q.dma_start(xv_t[:], xv.ap()[kt * P : (kt + 1) * P, :])
            for st in range(8):
                nc.tensor.matmul(
                    vacc[st],
                    xv_t[:, st * P : (st + 1) * P],
                    wv_t[:, :],
                    start=(kt == 0),
                    stop=(kt == DL // P - 1),
                )
        with nc.allow_low_precision(reason="bf16 v keeps ~8 mantissa bits"):
            for st in range(8):
                nc.vector.tensor_copy(
                    v_sb[:, st, :, 0:E],
                    vacc[st].rearrange("p (h e) -> p h e", e=E),
                )

    if not do_ab:
        # attention-only ablation: init residents so Tile sees writers
        # (memsets distribute across engines; ~upper-bounds attention span)
        for t_ in (kT, v_sb, qT, xt_sb, wq_sb, wo_sb):
            nc.vector.memset(t_[:], 0.01)

    if not do_attn:
        return

    # ---------- Phase C: attention, slab pipeline ----------
    # Unit of work = one score slab (lc, hp, st, a): [128 s, 512 l] for one
    # head-half.  Slabs stream st-major within each (lc, hp) unit; groups of
    # 3 consecutive slabs share one [P,1536] sc tile (3 banks) and ONE exp.
    # sc rotates x2 (6 banks) so exp(k+1) never waits on bank recycling:
    # ACT runs back-to-back.  AV is col-tiled (M=64, heads stacked on PSUM
    # partitions) into a 1-bank av tile; softmax denominators come from DVE
    # slab-sums (fp32) reduced+broadcast by a ones-stationary matmul.
    oTs = {}
    av_tiles = {}
    esums = {}

    def issue_scores(slabs, expS):
        sc_t = psum.tile([P, 1024], F32, tag="sc", name=uid("sc"))
        for i, (lc, hp, st, a) in enumerate(slabs):
            pa = 64 * a
            nc.tensor.matmul(
                sc_t[:, 512 * i : 512 * (i + 1)],
                kT[pa : pa + 64, hp, st * P : (st + 1) * P],
                qT[pa : pa + 64, hp, lc, :],
                start=True,
                stop=True,
            )
        with nc.allow_low_precision(reason="bf16 probs keep ~8 mantissa "
                                    "bits; tolerance 2e-2"):
            nc.scalar.activation(
                expS[:, 0 : len(slabs), :],
                sc_t[:, 0 : 512 * len(slabs)].rearrange(
                    "p (t l) -> p t l", l=LC
                ),
                mybir.ActivationFunctionType.Exp,
            )

    def issue_av(slabs, expS):
        for i, (lc, hp, st, a) in enumerate(slabs):
            if st == 0 and a == 0:
                av_tiles[(lc, hp)] = psumav.tile(
                    [P, 512], F32, tag="av", name=uid("av")
                )
                esums[(lc, hp)] = [
                    esump.tile([P, 512], F32, tag=f"es{j}", name=uid("es"))
                    for j in range(2)
                ]
            av = av_tiles[(lc, hp)]
            h = 2 * hp + a
            # col-tiled AV: head a -> PSUM partitions [64a, 64a+64).
            # start=True per head's first MM: the has_written clear applies
            # to the addressed region only (the baseline's two same-bank
            # broadcast MMs with start=True relied on exactly this).
            nc.tensor.matmul(
                av[64 * a : 64 * a + 64, :],
                v_sb[:, st, h, :],
                expS[:, i, :],
                start=(st == 0),
                stop=(st == 7),
                skip_group_check=True,
            )
            # denominator partial: esum_a += slab (fp32 accum on DVE)
            es = esums[(lc, hp)][a]
            if st == 0:
                nc.vector.tensor_copy(es[:], expS[:, i, :])
            else:
                nc.vector.tensor_tensor(
                    es[:], es[:], expS[:, i, :], mybir.AluOpType.add
                )
            if st == 7 and a == 1:
                finalize_pre(lc, hp)

    fin_pend = []  # (lc, hp, [esb0, esb1]) awaiting their PE stage

    def finalize_pre(lc, hp):
        # DVE-only stage, issued inline at (st7, a1): cast the fp32
        # denominator sums to bf16 for the matmul moving operand.
        es = esums.pop((lc, hp))
        esbs = []
        for a in range(2):
            esb = work.tile([P, LC], BF16, tag="esb", name=uid("esb"))
            with nc.allow_low_precision(reason="bf16 denominator; tol 2e-2"):
                nc.vector.tensor_copy(esb[:], es[a][:])
            esbs.append(esb)
        fin_pend.append((lc, hp, esbs))

    def finalize_mm(lc, hp, esbs):
        # PE + DVE stage, deferred ~2 groups so the pd matmuls never sit in
        # the PE FIFO waiting on the DVE casts (PE is strict FIFO: a waiting
        # matmul blocks the next groups' score matmuls behind it).
        if lc not in oTs:
            oTs[lc] = work.tile([P, 4, LC], BF16, tag="oT", name=uid("oT"))
        oT = oTs[lc]
        av = av_tiles.pop((lc, hp))
        pd = psum2.tile([P, 512], F32, tag="misc", name=uid("pd"))
        for a in range(2):
            nc.tensor.matmul(
                pd[64 * a : 64 * a + 64, :], ones128[:], esbs[a][:],
                start=True, stop=True,
            )
        bsb = work.tile([P, LC], BF16, tag="bsb", name=uid("bsb"))
        with nc.allow_low_precision(reason="denominator reciprocal in "
                                    "bf16; tolerance 2e-2"):
            nc.vector.reciprocal(bsb[:], pd[:])
            for a in range(2):
                nc.vector.tensor_mul(
                    oT[64 * a : 64 * a + 64, hp, :],
                    av[64 * a : 64 * a + 64, :],
                    bsb[64 * a : 64 * a + 64, :],
                )

    def outproj_group(lc, mt8):
        oT = oTs[lc]
        og = psum2.tile([P, 512], F32, tag="misc", name=uid("og"))
        for kt4 in range(4):
            nc.tensor.matmul(
                og[:],
                wo_sb[:, kt4, mt8 * P : (mt8 + 1) * P],
                oT[:, kt4, :],
                start=(kt4 == 0),
                stop=(kt4 == 3),
            )
        stg = work.tile([P, LC], BF16, tag="stg", name=uid("stg"))
        with nc.allow_low_precision(reason="bf16 partial output; host sums "
                                    "in fp32; tolerance 2e-2"):
            nc.vector.tensor_copy(stg[:], og[:])
        nc.sync.dma_start(
            out.ap()[mt8 * P : (mt8 + 1) * P, lc * LC : (lc + 1) * LC], stg[:]
        )
        if mt8 == 7:
            del oTs[lc]

    # slab stream: st-major within each (lc, hp) unit so each head's AV
    # accumulation sees st in order 0..7.
    slabs = [
        (lc, hp, st, a)
        for lc in range(N_LC)
        for hp in range(4)
        for st in range(8)
        for a in range(2)
    ]
    groups = [slabs[i : i + 2] for i in range(0, len(slabs), 2)]

    def fillers_for(slabs_done):
        # filler schedule keyed on lc-relative slab index (64 slabs per lc):
        # 4 Q-granules for lc+1 early in the lc, 8 out-proj groups for lc-1
        # spread across it (same cadence as the old per-batch schedule).
        for (lc, hp, st, a) in slabs_done:
            rel64 = hp * 16 + st * 2 + a
            if lc < N_LC - 1 and rel64 in (4, 12, 20, 28):
                yield ("q", lc + 1, (rel64 // 4 - 1) // 2)
            if lc > 0 and rel64 in (8, 16, 24, 32, 40, 48, 56, 60):
                yield ("o", lc - 1,
                       (8, 16, 24, 32, 40, 48, 56, 60).index(rel64))

    from collections import deque

    pend = deque()  # (slabs, expS), depth 2
    for gi, grp in enumerate(groups):
        expS = expp.tile([P, 2, LC], BF16, tag="expS", name=uid("e"))
        issue_scores(grp, expS)
        if len(pend) == 2:
            dslabs, dexp = pend.popleft()
            issue_av(dslabs, dexp)
            if len(fin_pend) > 1 or (fin_pend and gi % 2 == 0):
                finalize_mm(*fin_pend.pop(0))
            for kind, lc_, i_ in fillers_for(dslabs):
                if kind == "q":
                    q_granule(lc_, i_)
                else:
                    outproj_group(lc_, i_)
        pend.append((grp, expS))
    while pend:
        dslabs, dexp = pend.popleft()
        issue_av(dslabs, dexp)
        for kind, lc_, i_ in fillers_for(dslabs):
            if kind == "q":
                q_granule(lc_, i_)
            else:
                outproj_group(lc_, i_)
    while fin_pend:
        finalize_mm(*fin_pend.pop(0))
    for mt8 in range(8):
        outproj_group(N_LC - 1, mt8)


_NC_CACHE = {}


def _get_nc(reps=1, timing=False):
    if (reps, timing) not in _NC_CACHE:
        _NC_CACHE[(reps, timing)] = build_nc(reps, timing)
    return _NC_CACHE[(reps, timing)]


def make_in_maps(inputs):
    bf = ml_dtypes.bfloat16
    te = np.asarray(inputs["target_embedding"], np.float32)
    se = np.asarray(inputs["source_embedding"], np.float32)
    ve = np.asarray(inputs["value_embedding"], np.float32)
    Wq = np.asarray(inputs["Wq"], np.float32)
    Wk = np.asarray(inputs["Wk"], np.float32)
    Wv = np.asarray(inputs["Wv"], np.float32)
    Wo = np.asarray(inputs["Wo"], np.float32)
    bqv = np.asarray(inputs["bq"], np.float32)
    in_maps = []
    for core in range(8):
        b, g = divmod(core, 2)
        sl = slice(MQ * g, MQ * (g + 1))
        in_maps.append(
            {
                "xt": np.ascontiguousarray(te[b].T).astype(bf),
                "xs": np.ascontiguousarray(se[b].T).astype(bf),
                "xv": np.ascontiguousarray(ve[b].T).astype(bf),
                "wq": np.ascontiguousarray(Wq[:, sl]).astype(bf),
                "wk": np.ascontiguousarray(Wk[:, sl]).astype(bf),
                "wv": np.ascontiguousarray(Wv[:, sl]).astype(bf),
                "wo": np.ascontiguousarray(Wo[sl, :]).astype(bf),
                "bq": np.ascontiguousarray((bqv[sl] * SCALE).reshape(4, P).T),
            }
        )
    return in_maps


def assemble_output(results, inputs):
    bv = np.asarray(inputs["bv"], np.float32)
    bo = np.asarray(inputs["bo"], np.float32)
    Wo = np.asarray(inputs["Wo"], np.float32)
    corr = (bv @ Wo + bo).astype(np.float32)  # [D]
    out = np.empty((4, L, D), np.float32)
    for b in range(4):
        acc = results[2 * b]["out"].astype(np.float32) + results[
            2 * b + 1
        ]["out"].astype(np.float32)  # [D, L]
        out[b] = acc.T + corr
    return out


def _run_once(nc, in_maps, inputs):
    last_err = None
    for _attempt in range(3):
        try:
            res = run_bass_kernel_spmd(nc, in_maps, core_ids=list(range(8)))
            return assemble_output(res.results, inputs)
        except Exception as e:  # transient NRT device errors: retry
            last_err = e
    raise last_err


def kernel(**inputs) -> np.ndarray:
    nc = _get_nc(1)
    in_maps = make_in_maps(inputs)
    # Run twice and require agreement: guards against rare transient silent
    # device corruption (observed once after an abnormal device state).
    outs = [_run_once(nc, in_maps, inputs) for _ in range(2)]
    for _extra in range(2):
        scale = float(np.abs(outs[-1]).mean()) + 1e-30
        if any(
            np.abs(o - outs[-1]).max() < 1e-3 * scale for o in outs[:-1]
        ):
            return outs[-1]
        outs.append(_run_once(nc, in_maps, inputs))
    return outs[-1]


# revision 26
# speedup vs baseline: 1.2887x; 1.0267x over previous
"""Trainium2 Bass kernel for nn_AttentionLayer_13383118095164.

Cross-attention layer: q = target @ Wq + bq; k/v = source/value @ Wk/Wv + bk/bv;
out = softmax(q k^T / 8) v @ Wo + bo.   B=4, L=2048, S=1024, D=1024, H=16, E=64.

Sharding (8 cores): core c = (batch b = c//2, head-group g = c%2 of 8 heads).
Megatron-style: Q/K/V column-split by head group, Wo row-split; the two
head-group partial outputs per batch are summed on the host.

v6 design notes (session journey: 476us baseline -> 449 -> 434/424 (v3)
-> 406us; every rev driven by HW microbenches / phase ablation):
  * DMA: one HWDGE queue sustains only ~148GB/s (measured); the ~30MB of
    input traffic is split across the two parallel HWDGE queues (SP +
    Activation).  Projection phases ablate to ~115us/rep (DMA-bound).
  * Attention is a slab pipeline: slab = (lc, hp, st, head-half a) score
    matmul [128s x 512l]; pairs of slabs share one 2-bank sc tile and ONE
    exp (rotation x2 so exp never waits on bank recycling).  Scores go
    FIRST in each group's PE program; AV trails 2 groups; Q-proj/out-proj
    fillers ride per-lc slots.
  * AV is col-tiled (M=64, heads stacked on PSUM partitions, 1 bank per
    (lc,hp)); softmax denominators: DVE accumulates fp32 slab sums, a
    ones-stationary matmul reduces over s AND broadcasts per head, then
    reciprocal + multiply.  (The v1-v3 ones-column-in-V / M=65 trick was
    dropped to free the bank.)
  * PSUM: sc 2x2 + av 2x1 + misc 2x1 = 8 banks.  Phase ablation showed
    attention is latency-chain bound (306us for ~125us of ACT work with
    single-buffered av/misc): double-buffering av and misc removed the
    unit-boundary and filler-chain PE stalls and measured faster than
    bigger-but-single-buffered exp tiles (406 vs 424-434).
  * start=True on a matmul clears has_written only for the ADDRESSED
    region, not the whole bank (two same-bank partition-half accumulation
    groups each carry their own start=True; verified on HW).
  * Q-proj copyback on DVE (tensor_scalar); out written bf16; timing loop
    unrolled 4x per For_i iteration (boundary costs ~2.9us).
  Rejected after measuring slower: DVE evacuation of scores to SBUF for
  cheaper SBUF-sourced exp (490us); deeper stream/work pool bufs (470us);
  3-slab sc tiles with single-buffered av/misc (430us).
  Device layout otherwise as v1: everything transposed, bk dropped
  (softmax shift invariance), bv@Wo+bo folded into a host constant.
"""

import numpy as np
import ml_dtypes

import concourse.bass as bass
import concourse.tile as tile
from concourse import mybir
from concourse.bass_utils import run_bass_kernel_spmd

P = 128
D = 1024  # d_model
DL = 4096  # d_llm
L = 2048  # target length
S = 1024  # source length
MQ = 512  # per-core q/k/v dims (8 heads x 64)
E = 64
E1 = E + 1
HG = 8  # heads per core
LC = 512  # l-chunk
N_LC = L // LC
SCALE = 0.125  # 1/sqrt(E)

BF16 = mybir.dt.bfloat16
F32 = mybir.dt.float32


def _split_multi_waits(nc):
    """This walrus build rejects >1 sync wait per instruction: split extras
    onto single-wait NOPs on the same engine immediately before (same program
    order on the same queue => identical semantics)."""
    for f in nc.m.functions:
        for blk in f.blocks:
            new_insts = []
            for inst in blk.instructions:
                si = inst.sync_info
                if si is not None and si.on_wait and len(si.on_wait) > 1:
                    waits = list(si.on_wait)
                    for w in waits[:-1]:
                        nop = mybir.InstNoOp(
                            name=f"I-waitsplit-{nc.next_id()}", ins=[], outs=[]
                        )
                        nop.engine = inst.engine
                        nop.sync_info = mybir.SyncInfo(on_wait=[w], on_update=[])
                        new_insts.append(nop)
                    si.on_wait = [waits[-1]]
                new_insts.append(inst)
            blk.instructions[:] = new_insts


def build_nc(reps: int = 1, timing: bool = False, phases: str = "all"):
    """timing=True: identical device program, but all real tensors are
    Internal DRAM (uninitialized) with tiny dummy external I/O, so timing
    runs ship no host data and the slope measurement is low-variance. The
    engines here are data-independent in speed, so per-rep time is the same."""
    from contextlib import ExitStack

    nc = bass.Bass(trn_type="TRN2", target_bir_lowering=False, debug=False)

    ikind = "Internal" if timing else "ExternalInput"
    okind = "Internal" if timing else "ExternalOutput"
    xt = nc.dram_tensor("xt", [D, L], BF16, kind=ikind)  # X_t^T
    xs = nc.dram_tensor("xs", [DL, S], BF16, kind=ikind)  # X_s^T
    xv = nc.dram_tensor("xv", [DL, S], BF16, kind=ikind)  # X_v^T
    wq = nc.dram_tensor("wq", [D, MQ], BF16, kind=ikind)
    wk = nc.dram_tensor("wk", [DL, MQ], BF16, kind=ikind)
    wv = nc.dram_tensor("wv", [DL, MQ], BF16, kind=ikind)
    wo = nc.dram_tensor("wo", [MQ, D], BF16, kind=ikind)
    bq = nc.dram_tensor("bq", [P, 4], F32, kind=ikind)  # bq/8 as [p, mt]
    out = nc.dram_tensor("out", [D, L], BF16, kind=okind)  # out^T partial
    if timing:
        dummy = nc.dram_tensor("tdin", [1, 4], F32, kind="ExternalInput")
        dumout = nc.dram_tensor("tdout", [1, 4], F32, kind="ExternalOutput")

    with tile.TileContext(nc) as tc, ExitStack() as ctx:
        const = ctx.enter_context(tc.tile_pool(name="const", bufs=1))
        resident = ctx.enter_context(tc.tile_pool(name="resident", bufs=1))
        stream = ctx.enter_context(tc.tile_pool(name="stream", bufs=6))
        stream2 = ctx.enter_context(tc.tile_pool(name="stream2", bufs=6))
        psum = ctx.enter_context(tc.tile_pool(name="psum", bufs=2, space="PSUM"))
        psumav = ctx.enter_context(tc.tile_pool(name="psumav", bufs=2, space="PSUM"))
        psum2 = ctx.enter_context(tc.tile_pool(name="psum2", bufs=2, space="PSUM"))

        # ---- constants ----
        if timing:
            dtile = const.tile([1, 4], F32, name="dtile")
            nc.sync.dma_start(dtile[:], dummy.ap())
            nc.sync.dma_start(dumout.ap(), dtile[:])
        bq_sb = const.tile([P, 4], F32, name="bq_sb")
        nc.sync.dma_start(bq_sb[:], bq.ap())
        ones128 = const.tile([P, E], BF16, name="ones128")
        nc.vector.memset(ones128[:], 1.0)

        # ---- residents ----
        kT = resident.tile([P, 4, S], BF16, name="kT")  # [p, mt, s]
        v_sb = resident.tile([P, 8, HG, E], BF16, name="v_sb")  # [p, st, h, e]
        esump = ctx.enter_context(tc.tile_pool(name="esump", bufs=4))
        expp = ctx.enter_context(tc.tile_pool(name="expp", bufs=6))
        qT = resident.tile([P, 4, N_LC, LC], BF16, name="qT")  # [p, mt, lc, l]
        xt_sb = resident.tile([P, D // P, L], BF16, name="xt_sb")  # [p, kt, l]
        wq_sb = resident.tile([P, D // P, MQ], BF16, name="wq_sb")
        wo_sb = resident.tile([P, MQ // P, D], BF16, name="wo_sb")

        def _body(work):
            _emit_rep(
                nc, psum, psumav, psum2, esump, expp, work, stream, stream2,
                xt, xs, xv, wq, wk, wv, wo,
                bq_sb, ones128, kT, v_sb, qT, xt_sb, wq_sb, wo_sb, out,
                phases,
            )

        if reps == 1:
            with tc.tile_pool(name="work", bufs=4) as work:
                _body(work)
        else:
            # Unroll reps per For_i iteration: hardware-loop iterations
            # cannot overlap, but Tile schedules across the unrolled bodies,
            # so rep boundaries inside an iteration pipeline.
            unroll = 4 if reps % 4 == 0 else 2
            assert reps % 2 == 0 or reps == 1, "reps must be even"
            with tc.For_i(0, reps // unroll, 1):
                with tc.tile_pool(name="work", bufs=4) as work:
                    for _ in range(unroll):
                        _body(work)

    _split_multi_waits(nc)
    return nc


def _emit_rep(nc, psum, psumav, psum2, esump, expp, work, stream, stream2,
              xt, xs, xv, wq, wk, wv, wo,
              bq_sb, ones128, kT, v_sb, qT, xt_sb, wq_sb, wo_sb, out,
              phases="all"):
    names = [0]
    do_ab = phases in ("all", "ab")
    do_attn = phases in ("all", "attn")

    def uid(s):
        names[0] += 1
        return f"{s}_{names[0]}"

    def alloc8(pfx):
        """All 8 PSUM banks as 8 [P,512] accumulators (2 sc + av + misc)."""
        b0 = psum.tile([P, 1024], F32, tag="sc", name=uid(f"{pfx}b0"))
        b1 = psum.tile([P, 1024], F32, tag="sc", name=uid(f"{pfx}b1"))
        a0 = psumav.tile([P, 512], F32, tag="av", name=uid(f"{pfx}a0"))
        a1 = psumav.tile([P, 512], F32, tag="av", name=uid(f"{pfx}a1"))
        m0 = psum2.tile([P, 512], F32, tag="misc", name=uid(f"{pfx}m0"))
        m1 = psum2.tile([P, 512], F32, tag="misc", name=uid(f"{pfx}m1"))
        return [
            b0[:, 0:512], b0[:, 512:1024],
            b1[:, 0:512], b1[:, 512:1024],
            a0[:], a1[:],
            m0[:], m1[:],
        ]

    def q_granule(lc, mt):
        qm = psum2.tile([P, 512], F32, tag="misc", name=uid("qm"))
        for kt in range(D // P):
            nc.tensor.matmul(
                qm[:],
                wq_sb[:, kt, mt * P : (mt + 1) * P],
                xt_sb[:, kt, lc * LC : (lc + 1) * LC],
                start=(kt == 0),
                stop=(kt == D // P - 1),
            )
        with nc.allow_low_precision(reason="bf16 q keeps ~8 mantissa bits"):
            nc.vector.tensor_scalar(
                qT[:, mt, lc, :],
                qm[:],
                SCALE,
                bq_sb[:, mt : mt + 1],
                mybir.AluOpType.mult,
                mybir.AluOpType.add,
            )

    # ---------- Phase A: K^T = Wk_g^T @ X_s^T -> kT[p, mt, s] ----------
    # wk + residents ride the SP queue; xs streams on the ACT HWDGE queue
    # (idle during projections) so the two queues halve the DMA wall time.
    if do_ab:
        kacc = alloc8("k")
        for kt in range(DL // P):
            wk_t = stream.tile([P, MQ], BF16, tag="wk_t", name=uid("wk_t"))
            nc.sync.dma_start(wk_t[:], wk.ap()[kt * P : (kt + 1) * P, :])
            xs_t = stream2.tile([P, S], BF16, tag="xs_t", name=uid("xs_t"))
            nc.scalar.dma_start(xs_t[:], xs.ap()[kt * P : (kt + 1) * P, :])
            if kt >= 2 and kt % 3 == 2 and (kt - 2) // 3 < 8:
                j = (kt - 2) // 3
                nc.sync.dma_start(xt_sb[:, j, :], xt.ap()[j * P : (j + 1) * P, :])
            elif kt == 27:
                nc.sync.dma_start(
                    wq_sb[:], wq.ap().rearrange("(kt p) m -> p kt m", p=P)
                )
            elif kt == 30:
                nc.sync.dma_start(
                    wo_sb[:], wo.ap().rearrange("(kt p) d -> p kt d", p=P)
                )
            for mt in range(4):
                for sc in range(2):
                    nc.tensor.matmul(
                        kacc[mt * 2 + sc],
                        wk_t[:, mt * P : (mt + 1) * P],
                        xs_t[:, sc * 512 : (sc + 1) * 512],
                        start=(kt == 0),
                        stop=(kt == DL // P - 1),
                    )
        with nc.allow_low_precision(reason="bf16 operands keep ~8 mantissa "
                                    "bits; rel tolerance is 2e-2"):
            for i in range(8):
                nc.vector.tensor_copy(
                    kT[:, i // 2, (i % 2) * 512 : (i % 2 + 1) * 512], kacc[i]
                )

        # ---------- Phase Q0: Q proj for lc=0 ----------
        for mt in range(4):
            q_granule(0, mt)

        # ---------- Phase B: V = X_v @ Wv_g -> v_sb[p, st, h, 0:64] --------
        vacc = alloc8("v")
        for kt in range(DL // P):
            wv_t = stream.tile([P, MQ], BF16, tag="wk_t", name=uid("wv_t"))
            nc.sync.dma_start(wv_t[:], wv.ap()[kt * P : (kt + 1) * P, :])
            xv_t = stream2.tile([P, S], BF16, tag="xs_t", name=uid("xv_t"))
            # alternate xv between the two HWDGE queues to balance load
            xv_q = nc.scalar if kt % 2 == 0 else nc.sync
            xv_q.dma_start(xv_t[:], xv.ap()[kt * P : (kt + 1) * P, :])
            for st in range(8):
                nc.tensor.matmul(
                    vacc[st],
                    xv_t[:, st * P : (st + 1) * P],
                    wv_t[:, :],
                    start=(kt == 0),
                    stop=(kt == DL // P - 1),
                )
        with nc.allow_low_precision(reason="bf16 v keeps ~8 mantissa bits"):
            for st in range(8):
                nc.vector.tensor_copy(
                    v_sb[:, st, :, 0:E],
                    vacc[st].rearrange("p (h e) -> p h e", e=E),
                )

    if not do_ab:
        # attention-only ablation: init residents so Tile sees writers
        # (memsets distribute across engines; ~upper-bounds attention span)
        for t_ in (kT, v_sb, qT, xt_sb, wq_sb, wo_sb):
            nc.vector.memset(t_[:], 0.01)

    if not do_attn:
        return

    # ---------- Phase C: attention, slab pipeline ----------
    # Unit of work = one score slab (lc, hp, st, a): [128 s, 512 l] for one
    # head-half.  Slabs stream st-major within each (lc, hp) unit; groups of
    # 3 consecutive slabs share one [P,1536] sc tile (3 banks) and ONE exp.
    # sc rotates x2 (6 banks) so exp(k+1) never waits on bank recycling:
    # ACT runs back-to-back.  AV is col-tiled (M=64, heads stacked on PSUM
    # partitions) into a 1-bank av tile; softmax denominators come from DVE
    # slab-sums (fp32) reduced+broadcast by a ones-stationary matmul.
    oTs = {}
    av_tiles = {}
    esums = {}

    def issue_scores(slabs, expS):
        sc_t = psum.tile([P, 1024], F32, tag="sc", name=uid("sc"))
        for i, (lc, hp, st, a) in enumerate(slabs):
            pa = 64 * a
            nc.tensor.matmul(
                sc_t[:, 512 * i : 512 * (i + 1)],
                kT[pa : pa + 64, hp, st * P : (st + 1) * P],
                qT[pa : pa + 64, hp, lc, :],
                start=True,
                stop=True,
            )
        with nc.allow_low_precision(reason="bf16 probs keep ~8 mantissa "
                                    "bits; tolerance 2e-2"):
            nc.scalar.activation(
                expS[:, 0 : len(slabs), :],
                sc_t[:, 0 : 512 * len(slabs)].rearrange(
                    "p (t l) -> p t l", l=LC
                ),
                mybir.ActivationFunctionType.Exp,
            )

    def issue_av(slabs, expS):
        for i, (lc, hp, st, a) in enumerate(slabs):
            if st == 0 and a == 0:
                av_tiles[(lc, hp)] = psumav.tile(
                    [P, 512], F32, tag="av", name=uid("av")
                )
                esums[(lc, hp)] = [
                    esump.tile([P, 512], F32, tag=f"es{j}", name=uid("es"))
                    for j in range(2)
                ]
            av = av_tiles[(lc, hp)]
            h = 2 * hp + a
            # col-tiled AV: head a -> PSUM partitions [64a, 64a+64).
            # start=True per head's first MM: the has_written clear applies
            # to the addressed region only (the baseline's two same-bank
            # broadcast MMs with start=True relied on exactly this).
            nc.tensor.matmul(
                av[64 * a : 64 * a + 64, :],
                v_sb[:, st, h, :],
                expS[:, i, :],
                start=(st == 0),
                stop=(st == 7),
                skip_group_check=True,
            )
            # denominator partial: esum_a += slab (fp32 accum on DVE)
            es = esums[(lc, hp)][a]
            if st == 0:
                nc.vector.tensor_copy(es[:], expS[:, i, :])
            else:
                nc.vector.tensor_tensor(
                    es[:], es[:], expS[:, i, :], mybir.AluOpType.add
                )
            if st == 7 and a == 1:
                finalize_pre(lc, hp)

    fin_pend = []  # (lc, hp, [esb0, esb1]) awaiting their PE stage

    def finalize_pre(lc, hp):
        # DVE-only stage, issued inline at (st7, a1): cast the fp32
        # denominator sums to bf16 for the matmul moving operand.
        es = esums.pop((lc, hp))
        esbs = []
        for a in range(2):
            esb = work.tile([P, LC], BF16, tag="esb", name=uid("esb"))
            with nc.allow_low_precision(reason="bf16 denominator; tol 2e-2"):
                nc.vector.tensor_copy(esb[:], es[a][:])
            esbs.append(esb)
        fin_pend.append((lc, hp, esbs))

    def finalize_mm(lc, hp, esbs):
        # PE + DVE stage, deferred ~2 groups so the pd matmuls never sit in
        # the PE FIFO waiting on the DVE casts (PE is strict FIFO: a waiting
        # matmul blocks the next groups' score matmuls behind it).
        if lc not in oTs:
            oTs[lc] = work.tile([P, 4, LC], BF16, tag="oT", name=uid("oT"))
        oT = oTs[lc]
        av = av_tiles.pop((lc, hp))
        pd = psum2.tile([P, 512], F32, tag="misc", name=uid("pd"))
        for a in range(2):
            nc.tensor.matmul(
                pd[64 * a : 64 * a + 64, :], ones128[:], esbs[a][:],
                start=True, stop=True,
            )
        bsb = work.tile([P, LC], BF16, tag="bsb", name=uid("bsb"))
        with nc.allow_low_precision(reason="denominator reciprocal in "
                                    "bf16; tolerance 2e-2"):
            nc.vector.reciprocal(bsb[:], pd[:])
            for a in range(2):
                nc.vector.tensor_mul(
                    oT[64 * a : 64 * a + 64, hp, :],
                    av[64 * a : 64 * a + 64, :],
                    bsb[64 * a : 64 * a + 64, :],
                )

    def outproj_group(lc, mt8):
        oT = oTs[lc]
        og = psum2.tile([P, 512], F32, tag="misc", name=uid("og"))
        for kt4 in range(4):
            nc.tensor.matmul(
                og[:],
                wo_sb[:, kt4, mt8 * P : (mt8 + 1) * P],
                oT[:, kt4, :],
                start=(kt4 == 0),
                stop=(kt4 == 3),
            )
        stg = work.tile([P, LC], BF16, tag="stg", name=uid("stg"))
        with nc.allow_low_precision(reason="bf16 partial output; host sums "
                                    "in fp32; tolerance 2e-2"):
            nc.vector.tensor_copy(stg[:], og[:])
        nc.sync.dma_start(
            out.ap()[mt8 * P : (mt8 + 1) * P, lc * LC : (lc + 1) * LC], stg[:]
        )
        if mt8 == 7:
            del oTs[lc]

    # slab stream: st-major within each (lc, hp) unit so each head's AV
    # accumulation sees st in order 0..7.
    slabs = [
        (lc, hp, st, a)
        for lc in range(N_LC)
        for hp in range(4)
        for st in range(8)
        for a in range(2)
    ]
    groups = [slabs[i : i + 2] for i in range(0, len(slabs), 2)]

    def fillers_for(slabs_done):
        # filler schedule keyed on lc-relative slab index (64 slabs per lc):
        # 4 Q-granules for lc+1 early in the lc, 8 out-proj groups for lc-1
        # spread across it (same cadence as the old per-batch schedule).
        for (lc, hp, st, a) in slabs_done:
            rel64 = hp * 16 + st * 2 + a
            if lc < N_LC - 1 and rel64 in (4, 12, 20, 28):
                yield ("q", lc + 1, (rel64 // 4 - 1) // 2)
            if lc > 0 and rel64 in (8, 16, 24, 32, 40, 48, 56, 60):
                yield ("o", lc - 1,
                       (8, 16, 24, 32, 40, 48, 56, 60).index(rel64))

    from collections import deque

    pend = deque()  # (slabs, expS), depth 2
    for gi, grp in enumerate(groups):
        expS = expp.tile([P, 2, LC], BF16, tag="expS", name=uid("e"))
        issue_scores(grp, expS)
        if len(pend) == 2:
            dslabs, dexp = pend.popleft()
            issue_av(dslabs, dexp)
            if len(fin_pend) > 1 or (fin_pend and gi % 2 == 0):
                finalize_mm(*fin_pend.pop(0))
            for kind, lc_, i_ in fillers_for(dslabs):
                if kind == "q":
                    q_granule(lc_, i_)
                else:
                    outproj_group(lc_, i_)
        pend.append((grp, expS))
    while pend:
        dslabs, dexp = pend.popleft()
        issue_av(dslabs, dexp)
        for kind, lc_, i_ in fillers_for(dslabs):
            if kind == "q":
                q_granule(lc_, i_)
            else:
                outproj_group(lc_, i_)
    while fin_pend:
        finalize_mm(*fin_pend.pop(0))
    for mt8 in range(8):
        outproj_group(N_LC - 1, mt8)


_NC_CACHE = {}


def _get_nc(reps=1, timing=False):
    if (reps, timing) not in _NC_CACHE:
        _NC_CACHE[(reps, timing)] = build_nc(reps, timing)
    return _NC_CACHE[(reps, timing)]


def make_in_maps(inputs):
    bf = ml_dtypes.bfloat16
    te = np.asarray(inputs["target_embedding"], np.float32)
    se = np.asarray(inputs["source_embedding"], np.float32)
    ve = np.asarray(inputs["value_embedding"], np.float32)
    Wq = np.asarray(inputs["Wq"], np.float32)
    Wk = np.asarray(inputs["Wk"], np.float32)
    Wv = np.asarray(inputs["Wv"], np.float32)
    Wo = np.asarray(inputs["Wo"], np.float32)
    bqv = np.asarray(inputs["bq"], np.float32)
    in_maps = []
    for core in range(8):
        b, g = divmod(core, 2)
        sl = slice(MQ * g, MQ * (g + 1))
        in_maps.append(
            {
                "xt": np.ascontiguousarray(te[b].T).astype(bf),
                "xs": np.ascontiguousarray(se[b].T).astype(bf),
                "xv": np.ascontiguousarray(ve[b].T).astype(bf),
                "wq": np.ascontiguousarray(Wq[:, sl]).astype(bf),
                "wk": np.ascontiguousarray(Wk[:, sl]).astype(bf),
                "wv": np.ascontiguousarray(Wv[:, sl]).astype(bf),
                "wo": np.ascontiguousarray(Wo[sl, :]).astype(bf),
                "bq": np.ascontiguousarray((bqv[sl] * SCALE).reshape(4, P).T),
            }
        )
    return in_maps


def assemble_output(results, inputs):
    bv = np.asarray(inputs["bv"], np.float32)
    bo = np.asarray(inputs["bo"], np.float32)
    Wo = np.asarray(inputs["Wo"], np.float32)
    corr = (bv @ Wo + bo).astype(np.float32)  # [D]
    out = np.empty((4, L, D), np.float32)
    for b in range(4):
        acc = results[2 * b]["out"].astype(np.float32) + results[
            2 * b + 1
        ]["out"].astype(np.float32)  # [D, L]
        out[b] = acc.T + corr
    return out


def _run_once(nc, in_maps, inputs):
    last_err = None
    for _attempt in range(3):
        try:
            res = run_bass_kernel_spmd(nc, in_maps, core_ids=list(range(8)))
            return assemble_output(res.results, inputs)
        except Exception as e:  # transient NRT device errors: retry
            last_err = e
    raise last_err


def kernel(**inputs) -> np.ndarray:
    nc = _get_nc(1)
    in_maps = make_in_maps(inputs)
    # Run twice and require agreement: guards against rare transient silent
    # device corruption (observed once after an abnormal device state).
    outs = [_run_once(nc, in_maps, inputs) for _ in range(2)]
    for _extra in range(2):
        scale = float(np.abs(outs[-1]).mean()) + 1e-30
        if any(
            np.abs(o - outs[-1]).max() < 1e-3 * scale for o in outs[:-1]
        ):
            return outs[-1]
        outs.append(_run_once(nc, in_maps, inputs))
    return outs[-1]
